# revision 1
# baseline (speedup 1.0000x reference)
"""Trainium2 Bass kernel for LlamaMultiheadLatentAttention.

Contract: kernel(**inputs) takes FULL fp32 inputs (as produced by
reference.setup_inputs) and returns the FULL fp32 output [2, 1024, 4096].

Sharding (8 cores, no collectives): core c handles batch b = c//4 and
head-group g = c%4 (8 query heads, 2 kv heads, 8 latent heads). q/k/v and
latent projections are column-sharded per head-group; o_proj/latent_o_proj
are row-sharded, so each core emits a partial output sum and the host adds
the 4 partials per batch (the "all-reduce" of the output happens at unshard
time on the host).

Per-core kernel layout choices:
  - activations kept feature-major (xT, qT, kT, lkT, lqT: [feat_p, tokens])
    so every projection and attention matmul needs no transposes.
  - attention scores computed transposed, S^T[j, i] (k-tokens on partitions)
    which makes the softmax denominator a ones-vector matmul on TensorE and
    keeps P^T in exactly the layout the P@V matmul wants.
  - softmax without max-subtraction (scores are O(+-10) here; exp is safe in
    fp32) and causal masking by skipping fully-masked j-blocks plus a
    0/1-mask multiply on the 4 diagonal block patterns.
  - all matmul operands bf16 (4x TensorE throughput vs fp32), fp32 PSUM.
"""

import numpy as np
import ml_dtypes

import concourse.bass as bass
import concourse.mybir as mybir
import concourse.tile as tile
from concourse import bacc
from concourse.bass_utils import run_bass_kernel_spmd

BF16 = ml_dtypes.bfloat16

B, S, D = 2, 1024, 4096
H, KVH, HD = 32, 8, 128
GROUPS = H // KVH
LAT, LH = 1024, 32
THETA = 10000.0
SCALE = 1.0 / float(np.sqrt(HD))

NCORES = 8
TP = 4                 # head-group shards
HL = H // TP           # 8 local q heads
KVL = KVH // TP        # 2 local kv heads
LHL = LH // TP         # 8 local latent heads

f32 = mybir.dt.float32
bf16 = mybir.dt.bfloat16

D_T = D // 128         # 32 k-tiles over model dim
LAT_T = LAT // 128     # 8 k-tiles over latent dim
S_T = S // 128         # 8 token tiles of 128
IB = 2                 # token blocks of 512 (columns of feature-major mats)
JT = S // 128          # 8 j-tiles for attention


def _build_program():
    nc = bacc.Bacc("TRN2", target_bir_lowering=False, debug=False)

    xt_d = nc.dram_tensor("xt", [128, D_T, S], bf16, kind="ExternalInput")
    wq_d = nc.dram_tensor("wq", [HL, 128, D_T, 128], bf16, kind="ExternalInput")
    wk_d = nc.dram_tensor("wk", [KVL, 128, D_T, 128], bf16, kind="ExternalInput")
    wv_d = nc.dram_tensor("wv", [128, D_T, KVL * HD], bf16, kind="ExternalInput")
    wlq_d = nc.dram_tensor("wlq", [LAT_T, 128, D_T, 128], bf16, kind="ExternalInput")
    wlk_d = nc.dram_tensor("wlk", [LHL, 128, LAT_T, 128], bf16, kind="ExternalInput")
    wlv_d = nc.dram_tensor("wlv", [128, D_T, LHL * HD], bf16, kind="ExternalInput")
    wo_d = nc.dram_tensor("wo", [8, 128, HL, 512], bf16, kind="ExternalInput")
    wlo_d = nc.dram_tensor("wlo", [8, 128, LHL, 512], bf16, kind="ExternalInput")
    cos_d = nc.dram_tensor("cosT", [HD, S], f32, kind="ExternalInput")
    sin_d = nc.dram_tensor("sinTs", [HD, S], f32, kind="ExternalInput")
    mask_d = nc.dram_tensor("maskP", [128, 4, 512], bf16, kind="ExternalInput")
    out_d = nc.dram_tensor("out", [S, D], f32, kind="ExternalOutput")

    out_ap = out_d.ap().rearrange("(tt p) d -> p tt d", p=128)

    with tile.TileContext(nc) as tc:
        with tc.tile_pool(name="const", bufs=1) as constp, \
             tc.tile_pool(name="acts", bufs=1) as acts:

            cosT = constp.tile([HD, S], f32, tag="cosT")
            sinTs = constp.tile([HD, S], f32, tag="sinTs")
            maskP = constp.tile([128, 4, 512], bf16, tag="maskP")
            ones = constp.tile([128, 1], bf16, tag="ones")
            nc.sync.dma_start(cosT[:], cos_d.ap())
            nc.sync.dma_start(sinTs[:], sin_d.ap())
            nc.sync.dma_start(maskP[:], mask_d.ap())
            nc.vector.memset(ones[:], 1.0)

            # persistent activations (bf16)
            qT = acts.tile([128, HL, S], bf16, tag="qT")
            kT = acts.tile([128, KVL, S], bf16, tag="kT")
            lkT = acts.tile([128, LHL, S], bf16, tag="lkT")
            v_sb = acts.tile([128, S_T, KVL * HD], bf16, tag="v")
            lv_sb = acts.tile([128, S_T, LHL * HD], bf16, tag="lv")

            with tc.tile_pool(name="xt", bufs=1) as xtp:
                xt = xtp.tile([128, D_T, S], bf16, tag="xt")
                for c in range(4):
                    nc.sync.dma_start(
                        xt[:, bass.ts(c, D_T // 4), :],
                        xt_d.ap()[:, bass.ts(c, D_T // 4), :])

                # ---- phase B2: token-major projections v, lv ----
                # two passes over tokens; wlv streamed in 512-col halves
                with tc.tile_pool(name="wvlv", bufs=1) as wvp, \
                     tc.tile_pool(name="ps_b2", bufs=4, space="PSUM") as psb2:
                    wv_sb = wvp.tile([128, D_T, KVL * HD], bf16, tag="wv")
                    for c in range(4):
                        nc.sync.dma_start(
                            wv_sb[:, bass.ts(c, D_T // 4), :],
                            wv_d.ap()[:, bass.ts(c, D_T // 4), :])
                    for half in range(2):
                        wlv_sb = wvp.tile([128, D_T, 512], bf16, tag="wlvh",
                                          name=f"wlvh_{half}")
                        hs = bass.ts(half, 512)
                        for c in range(4):
                            nc.sync.dma_start(
                                wlv_sb[:, bass.ts(c, D_T // 4), :],
                                wlv_d.ap()[:, bass.ts(c, D_T // 4), hs])
                        for tp_ in range(S_T // 2):   # token-tile pairs
                            pss = []
                            for u in range(2):
                                pair = [psb2.tile(
                                    [128, 512], f32, tag="ps_lv",
                                    name=f"ps_lv_{half}_{tp_}_{u}")]
                                if half == 0:
                                    pair.append(psb2.tile(
                                        [128, KVL * HD], f32, tag="ps_v",
                                        name=f"ps_v_{tp_}_{u}"))
                                pss.append(pair)
                            for kt in range(D_T):
                                st = kt == 0
                                sp = kt == D_T - 1
                                for u in range(2):
                                    tt = 2 * tp_ + u
                                    lhs = xt[:, kt, bass.ts(tt, 128)]
                                    nc.tensor.matmul(pss[u][0][:], lhs,
                                                     wlv_sb[:, kt, :],
                                                     start=st, stop=sp)
                                    if half == 0:
                                        nc.tensor.matmul(pss[u][1][:], lhs,
                                                         wv_sb[:, kt, :],
                                                         start=st, stop=sp)
                            for u in range(2):
                                tt = 2 * tp_ + u
                                nc.any.tensor_copy(lv_sb[:, tt, hs],
                                                   pss[u][0][:])
                                if half == 0:
                                    nc.any.tensor_copy(v_sb[:, tt, :],
                                                       pss[u][1][:])

                # ---- phase B1: feature-major projections q, k, lq, lk (+rope)
                with tc.tile_pool(name="lq", bufs=1) as lqp, \
                     tc.tile_pool(name="wstr", bufs=3) as wstr, \
                     tc.tile_pool(name="rope", bufs=4) as ropep, \
                     tc.tile_pool(name="ps_b1", bufs=4, space="PSUM") as psb1:

                    lqT = lqp.tile([128, LAT_T, S], bf16, tag="lqT")

                    def rope_to(dst, ps, ib):
                        sl = bass.ts(ib, 512)
                        rt = ropep.tile([128, 512], f32, tag="rt")
                        qc = ropep.tile([128, 512], f32, tag="qc")
                        nc.vector.tensor_tensor(
                            rt[0:64, :], ps[64:128, :], sinTs[0:64, sl],
                            mybir.AluOpType.mult)
                        nc.vector.tensor_tensor(
                            rt[64:128, :], ps[0:64, :], sinTs[64:128, sl],
                            mybir.AluOpType.mult)
                        nc.vector.tensor_tensor(
                            qc[:], ps[:], cosT[:, sl], mybir.AluOpType.mult)
                        nc.vector.tensor_add(dst, qc[:], rt[:])

                    def proj_fm(w_dram, n_tiles, src, src_t, dst, rope):
                        # dst[:, nt, :] = (w[:, nt-block].T @ src), optionally roped
                        for nt in range(n_tiles):
                            wt = wstr.tile([128, src_t, 128], bf16,
                                           tag=f"w_{src_t}")
                            nc.sync.dma_start(wt[:], w_dram.ap()[nt])
                            ps = [psb1.tile([128, 512], f32, tag="ps_b1",
                                            name=f"ps_b1_{nt}_{ib}")
                                  for ib in range(IB)]
                            for kt in range(src_t):
                                for ib in range(IB):
                                    nc.tensor.matmul(
                                        ps[ib][:], wt[:, kt, :],
                                        src[:, kt, bass.ts(ib, 512)],
                                        start=(kt == 0), stop=(kt == src_t - 1))
                            for ib in range(IB):
                                dsl = dst[:, nt, bass.ts(ib, 512)]
                                if rope:
                                    rope_to(dsl, ps[ib][:], ib)
                                else:
                                    nc.any.tensor_copy(dsl, ps[ib][:])

                    proj_fm(wlq_d, LAT_T, xt, D_T, lqT, rope=False)
                    proj_fm(wq_d, HL, xt, D_T, qT, rope=True)
                    proj_fm(wk_d, KVL, xt, D_T, kT, rope=True)
                    proj_fm(wlk_d, LHL, lqT, LAT_T, lkT, rope=True)

            # ---- phase C: attention (16 virtual heads; scores transposed) ----
            with tc.tile_pool(name="attnlat", bufs=1) as alp:
                attnT = alp.tile([128, HL, S], bf16, tag="attnT")
                latT = alp.tile([128, LHL, S], bf16, tag="latT")

                with tc.tile_pool(name="pp", bufs=16) as pp, \
                     tc.tile_pool(name="dn", bufs=3) as dn, \
                     tc.tile_pool(name="ps_s", bufs=4, space="PSUM") as pss_, \
                     tc.tile_pool(name="ps_d", bufs=2, space="PSUM") as psd_, \
                     tc.tile_pool(name="ps_o", bufs=2, space="PSUM") as pso_:

                    for vh in range(HL + LHL):
                        if vh < HL:
                            h = vh
                            ksrc = kT[:, h // GROUPS, :]
                            dst = attnT
                        else:
                            h = vh - HL
                            ksrc = lkT[:, h, :]
                            dst = latT
                        qsrc = qT[:, h, :]

                        for ib in range(IB):
                            njb = 4 * (ib + 1)
                            isl = bass.ts(ib, 512)
                            pts = []
                            for jb in range(njb):
                                ps_s = pss_.tile([128, 512], f32, tag="ps_s")
                                nc.tensor.matmul(
                                    ps_s[:], ksrc[:, bass.ts(jb, 128)],
                                    qsrc[:, isl], start=True, stop=True)
                                pt = pp.tile([128, 512], bf16, tag="pt")
                                nc.scalar.activation(
                                    pt[:], ps_s[:],
                                    mybir.ActivationFunctionType.Exp,
                                    scale=SCALE)
                                r = jb - 4 * ib
                                if r >= 0:
                                    nc.vector.tensor_tensor(
                                        pt[:], pt[:], maskP[:, r, :],
                                        mybir.AluOpType.mult)
                                pts.append(pt)

                            ps_o = pso_.tile([128, 512], f32, tag="ps_o")
                            for jb in range(njb):
                                if vh < HL:
                                    vsl = v_sb[:, jb,
                                               bass.ts(h // GROUPS, HD)]
                                else:
                                    vsl = lv_sb[:, jb, bass.ts(h, HD)]
                                nc.tensor.matmul(
                                    ps_o[:], vsl, pts[jb][:],
                                    start=(jb == 0), stop=(jb == njb - 1))

                            ps_d = psd_.tile([1, 512], f32, tag="ps_d")
                            for jb in range(njb):
                                nc.tensor.matmul(
                                    ps_d[:], ones[:, :], pts[jb][:],
                                    start=(jb == 0), stop=(jb == njb - 1))
                            rec = dn.tile([1, 512], f32, tag="rec")
                            nc.vector.reciprocal(rec[:], ps_d[:])
                            recb = dn.tile([128, 512], f32, tag="recb")
                            nc.gpsimd.partition_broadcast(recb[:], rec[:])
                            nc.vector.tensor_tensor(
                                dst[:, h, isl], ps_o[:], recb[:],
                                mybir.AluOpType.mult)

                # ---- phase D: output projections (row-sharded, partial sum) --
                with tc.tile_pool(name="wop", bufs=2) as wop, \
                     tc.tile_pool(name="ost", bufs=4) as ost, \
                     tc.tile_pool(name="ps_f", bufs=4, space="PSUM") as psf:
                    for np_ in range(4):       # pairs of 512-wide col blocks
                        wo2 = wop.tile([128, HL, 1024], bf16, tag="wo2")
                        wlo2 = wop.tile([128, LHL, 1024], bf16, tag="wlo2")
                        for u in range(2):
                            nc.sync.dma_start(
                                wo2[:, :, bass.ts(u, 512)],
                                wo_d.ap()[2 * np_ + u])
                            nc.sync.dma_start(
                                wlo2[:, :, bass.ts(u, 512)],
                                wlo_d.ap()[2 * np_ + u])
                        for tt in range(S_T):
                            ps0 = psf.tile([128, 512], f32, tag="ps_f")
                            ps1 = psf.tile([128, 512], f32, tag="ps_f")
                            for h in range(HL):
                                lhs = attnT[:, h, bass.ts(tt, 128)]
                                nc.tensor.matmul(ps0[:], lhs,
                                                 wo2[:, h, 0:512],
                                                 start=(h == 0), stop=False)
                                nc.tensor.matmul(ps1[:], lhs,
                                                 wo2[:, h, 512:1024],
                                                 start=(h == 0), stop=False)
                            for h in range(LHL):
                                lhs = latT[:, h, bass.ts(tt, 128)]
                                nc.tensor.matmul(ps0[:], lhs,
                                                 wlo2[:, h, 0:512],
                                                 start=False, stop=(h == LHL - 1))
                                nc.tensor.matmul(ps1[:], lhs,
                                                 wlo2[:, h, 512:1024],
                                                 start=False, stop=(h == LHL - 1))
                            for u, ps in enumerate((ps0, ps1)):
                                ot = ost.tile([128, 512], f32, tag="ot")
                                nc.any.tensor_copy(ot[:], ps[:])
                                nc.sync.dma_start(
                                    out_ap[:, tt, bass.ds(
                                        (2 * np_ + u) * 512, 512)],
                                    ot[:])

    nc.compile()
    return nc


_NC = None


def _get_program():
    global _NC
    if _NC is None:
        _NC = _build_program()
    return _NC


def _rope_tables():
    inv_freq = 1.0 / (THETA ** (np.arange(0, HD, 2, dtype=np.float32) / HD))
    t = np.arange(S, dtype=np.float32)
    freqs = np.outer(t, inv_freq)                       # [S, 64]
    emb = np.concatenate([freqs, freqs], axis=-1)       # [S, HD]
    cosT = np.cos(emb).T.astype(np.float32).copy()      # [HD, S]
    sinT = np.sin(emb).T.astype(np.float32)
    sinTs = np.concatenate([-sinT[:HD // 2], sinT[HD // 2:]], 0).astype(
        np.float32).copy()
    return cosT, sinTs


def _mask_patterns():
    # maskP[p, r, i] = 1.0 iff (r*128 + p) <= i, for i in [0, 512)
    p = np.arange(128)[:, None, None]
    r = np.arange(4)[None, :, None]
    i = np.arange(512)[None, None, :]
    return ((r * 128 + p) <= i).astype(BF16)


def _tile_w_fm(w, n_tiles, kt):
    # [K, n_tiles*128] -> [n_tiles, 128(p of K), kt, 128]
    K, N = w.shape
    assert K == kt * 128 and N == n_tiles * 128
    return np.ascontiguousarray(
        w.reshape(kt, 128, n_tiles, 128).transpose(2, 1, 0, 3)).astype(BF16)


def _tile_w_tm(w, kt):
    # [K, N] -> [128(p of K), kt, N]
    K, N = w.shape
    assert K == kt * 128
    return np.ascontiguousarray(
        w.reshape(kt, 128, N).transpose(1, 0, 2)).astype(BF16)


def _tile_w_out(w):
    # [1024, D] -> [8(nb), 128(p of rows), 8(h), 512]
    return np.ascontiguousarray(
        w.reshape(8, 128, D // 512, 512).transpose(2, 1, 0, 3)).astype(BF16)


def kernel(hidden_states, w_q, w_k, w_v, w_o, w_lq, w_lk, w_lv, w_lo):
    nc = _get_program()
    cosT, sinTs = _rope_tables()
    maskP = _mask_patterns()

    in_maps = []
    for c in range(NCORES):
        b, g = divmod(c, TP)
        x = np.asarray(hidden_states[b], dtype=np.float32)       # [S, D]
        xt = np.ascontiguousarray(
            x.T.reshape(D_T, 128, S).transpose(1, 0, 2)).astype(BF16)
        qs = slice(g * HL * HD, (g + 1) * HL * HD)
        kvs = slice(g * KVL * HD, (g + 1) * KVL * HD)
        ls = slice(g * LHL * HD, (g + 1) * LHL * HD)
        in_maps.append({
            "xt": xt,
            "wq": _tile_w_fm(np.asarray(w_q)[:, qs], HL, D_T),
            "wk": _tile_w_fm(np.asarray(w_k)[:, kvs], KVL, D_T),
            "wv": _tile_w_tm(np.asarray(w_v)[:, kvs], D_T),
            "wlq": _tile_w_fm(np.asarray(w_lq), LAT_T, D_T),
            "wlk": _tile_w_fm(np.asarray(w_lk)[:, ls], LHL, LAT_T),
            "wlv": _tile_w_tm(np.asarray(w_lv)[:, ls], D_T),
            "wo": _tile_w_out(np.asarray(w_o)[qs, :]),
            "wlo": _tile_w_out(np.asarray(w_lo)[ls, :]),
            "cosT": cosT,
            "sinTs": sinTs,
            "maskP": maskP,
        })

    res = run_bass_kernel_spmd(nc, in_maps, list(range(NCORES))).results

    out = np.zeros((B, S, D), dtype=np.float32)
    for c in range(NCORES):
        b = c // TP
        out[b] += res[c]["out"]
    return out



# revision 7
# speedup vs baseline: 1.2415x; 1.2415x over previous
"""Trainium2 Bass kernel for LlamaMultiheadLatentAttention.

Contract: kernel(**inputs) takes FULL fp32 inputs (as produced by
reference.setup_inputs) and returns the FULL fp32 output [2, 1024, 4096].

Sharding (8 cores, no collectives): core c handles batch b = c//4 and
head-group g = c%4 (8 query heads, 2 kv heads, 8 latent heads). q/k/v and
latent projections are column-sharded per head-group; o_proj/latent_o_proj
are row-sharded, so each core emits a partial output sum and the host adds
the 4 partials per batch.

Key layout/optimization choices:
  - lk is computed as x @ (w_lq @ w_lk) with the weight product folded on
    the host, removing the duplicated latent-q projection entirely.
  - activations feature-major (xT, qT, kT, lkT: [feat_p, tokens]) so every
    projection and attention matmul needs no transposes.
  - attention scores computed transposed, S^T[k, q] (k-tokens on partitions);
    causal structure exploited by trimming diagonal j-blocks to their valid
    query range and packing the trimmed blocks tightly into PSUM banks so a
    single exp instruction covers contiguous valid data.
  - softmax denominator via an all-ones [128,128] stationary matmul: the
    denominator arrives already broadcast across partitions in PSUM; the
    reciprocal uses the fast approximate DVE op (~18 bits, plenty here).
  - per-head SBUF tiles (qT_h, lkT_h, attnT_h, latT_h) give the Tile
    scheduler fine-grained dependencies, so projection, attention, and
    output-projection phases overlap instead of serializing.
  - all matmul operands bf16 (4x TensorE throughput vs fp32), fp32 PSUM.
"""

import numpy as np
import ml_dtypes

import concourse.bass as bass
import concourse.mybir as mybir
import concourse.tile as tile
from concourse import bacc
from concourse.bass_utils import run_bass_kernel_spmd

BF16 = ml_dtypes.bfloat16

B, S, D = 2, 1024, 4096
H, KVH, HD = 32, 8, 128
GROUPS = H // KVH
LAT, LH = 1024, 32
THETA = 10000.0
SCALE = 1.0 / float(np.sqrt(HD))

NCORES = 8
TP = 4                 # head-group shards
HL = H // TP           # 8 local q heads
KVL = KVH // TP        # 2 local kv heads
LHL = LH // TP         # 8 local latent heads

f32 = mybir.dt.float32
bf16 = mybir.dt.bfloat16

D_T = D // 128         # 32 k-tiles over model dim
S_T = S // 128         # 8 token tiles of 128
IB = 2                 # token blocks of 512


def _attn_blocks(ib):
    """Causal block layout for query block ib (512 queries).

    Returns list of (jb, off, width, sc) where jb is the key tile, off the
    column offset inside the score-group PSUM tile, width the number of valid
    query columns, and sc the query-column start within the 512-block.
    Grouped so that each group is one PSUM tile ([128, 1024] max, each
    matmul output within a single 512-col bank) and the valid columns are
    contiguous from 0 (one exp covers them with no gaps).
    """
    groups = []
    full = [jb for jb in range(4 * ib)]          # non-diagonal: full width
    for pair in range(len(full) // 2):
        a, b_ = full[2 * pair], full[2 * pair + 1]
        groups.append(([(a, 0, 512, 0), (b_, 512, 512, 0)], 1024))
    dg = 4 * ib
    # diagonal blocks dg+0..dg+3 with widths 512,384,256,128
    groups.append(([(dg, 0, 512, 0), (dg + 1, 512, 384, 128),
                    (dg + 3, 896, 128, 384)], 1024))
    groups.append(([(dg + 2, 0, 256, 256)], 256))
    return groups


def _build_program():
    nc = bacc.Bacc("TRN2", target_bir_lowering=False, debug=False)

    xt_d = nc.dram_tensor("xt", [128, D_T, S], bf16, kind="ExternalInput")
    wq_d = nc.dram_tensor("wq", [HL, 128, D_T, 128], bf16, kind="ExternalInput")
    wk_d = nc.dram_tensor("wk", [KVL, 128, D_T, 128], bf16, kind="ExternalInput")
    wv_d = nc.dram_tensor("wv", [128, D_T, KVL * HD], bf16, kind="ExternalInput")
    wlkc_d = nc.dram_tensor("wlkc", [LHL, 128, D_T, 128], bf16,
                            kind="ExternalInput")
    wlv_d = nc.dram_tensor("wlv", [128, D_T, LHL * HD], bf16,
                           kind="ExternalInput")
    wo_d = nc.dram_tensor("wo", [8, 128, HL, 512], bf16, kind="ExternalInput")
    wlo_d = nc.dram_tensor("wlo", [8, 128, LHL, 512], bf16,
                           kind="ExternalInput")
    cos_d = nc.dram_tensor("cosT", [HD, S], bf16, kind="ExternalInput")
    sin_d = nc.dram_tensor("sinTs", [HD, S], bf16, kind="ExternalInput")
    tri_d = nc.dram_tensor("trimask", [128, 128], bf16, kind="ExternalInput")
    out_d = nc.dram_tensor("out", [S, D], f32, kind="ExternalOutput")

    out_ap = out_d.ap().rearrange("(tt p) d -> p tt d", p=128)

    with tile.TileContext(nc) as tc:
        with tc.tile_pool(name="const", bufs=1) as constp, \
             tc.tile_pool(name="acts", bufs=1) as acts:

            cosT = constp.tile([HD, S], bf16, tag="cosT")
            sinTs = constp.tile([HD, S], bf16, tag="sinTs")
            tri = constp.tile([128, 128], bf16, tag="tri")
            ones = constp.tile([128, 128], bf16, tag="ones")
            nc.sync.dma_start(cosT[:], cos_d.ap())
            nc.sync.dma_start(sinTs[:], sin_d.ap())
            nc.sync.dma_start(tri[:], tri_d.ap())
            nc.vector.memset(ones[:], 1.0)

            # persistent activations (bf16), per-head tiles for fine deps
            v_sb = acts.tile([128, S_T, KVL * HD], bf16, tag="v")
            lv_sb = acts.tile([128, S_T, LHL * HD], bf16, tag="lv")
            kT = [acts.tile([128, S], bf16, tag=f"kT{i}", name=f"kT{i}")
                  for i in range(KVL)]
            qT = [acts.tile([128, S], bf16, tag=f"qT{h}", name=f"qT{h}")
                  for h in range(HL)]
            lkT = [acts.tile([128, S], bf16, tag=f"lkT{h}", name=f"lkT{h}")
                   for h in range(LHL)]
            with tc.tile_pool(name="xt", bufs=1) as xtp:
                xt = [xtp.tile([128, D_T // 4, S], bf16, tag=f"xt{c}", name=f"xt{c}")
                      for c in range(4)]
                for c in range(4):
                    nc.sync.dma_start(
                        xt[c][:], xt_d.ap()[:, bass.ts(c, D_T // 4), :])

                def xts(kt, sl):
                    return xt[kt // 8][:, kt % 8, sl]

                # ---- phase B2: token-major projections v, lv ----
                with tc.tile_pool(name="wvp", bufs=1) as wvp, \
                     tc.tile_pool(name="wlvp", bufs=2) as wlvp, \
                     tc.tile_pool(name="ps_b2", bufs=4, space="PSUM") as psb2:
                    wv_sb = wvp.tile([128, D_T, KVL * HD], bf16, tag="wv")
                    for c in range(4):
                        nc.sync.dma_start(
                            wv_sb[:, bass.ts(c, D_T // 4), :],
                            wv_d.ap()[:, bass.ts(c, D_T // 4), :])
                    for half in range(2):
                        wlv_sb = wlvp.tile([128, D_T, 512], bf16, tag="wlvh",
                                          name=f"wlvh_{half}")
                        hs = bass.ts(half, 512)
                        for c in range(4):
                            nc.sync.dma_start(
                                wlv_sb[:, bass.ts(c, D_T // 4), :],
                                wlv_d.ap()[:, bass.ts(c, D_T // 4), hs])
                        for tp_ in range(S_T // 2):   # token-tile pairs
                            pss = []
                            for u in range(2):
                                pair = [psb2.tile(
                                    [128, 512], f32, tag="ps_lv",
                                    name=f"ps_lv_{half}_{tp_}_{u}")]
                                if half == 0:
                                    pair.append(psb2.tile(
                                        [128, KVL * HD], f32, tag="ps_v",
                                        name=f"ps_v_{tp_}_{u}"))
                                pss.append(pair)
                            for kt in range(D_T):
                                st = kt == 0
                                sp = kt == D_T - 1
                                for u in range(2):
                                    tt = 2 * tp_ + u
                                    lhs = xts(kt, bass.ts(tt, 128))
                                    nc.tensor.matmul(pss[u][0][:], lhs,
                                                     wlv_sb[:, kt, :],
                                                     start=st, stop=sp)
                                    if half == 0:
                                        nc.tensor.matmul(pss[u][1][:], lhs,
                                                         wv_sb[:, kt, :],
                                                         start=st, stop=sp)
                            for u in range(2):
                                tt = 2 * tp_ + u
                                nc.any.tensor_copy(lv_sb[:, tt, hs],
                                                   pss[u][0][:])
                                if half == 0:
                                    nc.any.tensor_copy(v_sb[:, tt, :],
                                                       pss[u][1][:])

                # ---- phase B1: feature-major projections q, k, lk (+rope) --
                with tc.tile_pool(name="wstr", bufs=3) as wstr, \
                     tc.tile_pool(name="rope", bufs=2) as ropep, \
                     tc.tile_pool(name="ps_b1", bufs=3, space="PSUM") as psb1:

                    def rope_to(dst, ps, ib):
                        sl = bass.ts(ib, 512)
                        rt = ropep.tile([128, 512], f32, tag="rt", name="rt")
                        qc = ropep.tile([128, 512], f32, tag="qc", name="qc")
                        nc.vector.tensor_tensor(
                            rt[0:64, :], ps[64:128, :], sinTs[0:64, sl],
                            mybir.AluOpType.mult)
                        nc.vector.tensor_tensor(
                            rt[64:128, :], ps[0:64, :], sinTs[64:128, sl],
                            mybir.AluOpType.mult)
                        nc.vector.tensor_tensor(
                            qc[:], ps[:], cosT[:, sl], mybir.AluOpType.mult)
                        nc.vector.tensor_add(dst, qc[:], rt[:])

                    def proj_head(w_dram, nt, dst):
                        # dst[:, :] = rope(w[nt].T @ xt)
                        wt = wstr.tile([128, D_T, 128], bf16, tag="w32",
                                       name=f"w_{w_dram.name}_{nt}")
                        nc.sync.dma_start(wt[:], w_dram.ap()[nt])
                        ps = [psb1.tile([128, 512], f32, tag="ps_b1",
                                        name=f"ps_{w_dram.name}_{nt}_{ib}")
                              for ib in range(IB)]
                        for kt in range(D_T):
                            for ib in range(IB):
                                nc.tensor.matmul(
                                    ps[ib][:], wt[:, kt, :],
                                    xts(kt, bass.ts(ib, 512)),
                                    start=(kt == 0), stop=(kt == D_T - 1))
                        for ib in range(IB):
                            rope_to(dst[:, bass.ts(ib, 512)], ps[ib][:], ib)

                    for i in range(KVL):
                        proj_head(wk_d, i, kT[i])
                    for h in range(HL):
                        proj_head(wq_d, h, qT[h])
                    for h in range(LHL):
                        proj_head(wlkc_d, h, lkT[h])

            # ---- phase C: attention (16 virtual heads, causal-trimmed) ----
            # ---- phase D: output projections (overlapped via scheduler) ---
            with tc.tile_pool(name="attnlat", bufs=1) as alp, \
                 tc.tile_pool(name="pt", bufs=4) as ptp, \
                 tc.tile_pool(name="rec", bufs=2) as recp, \
                 tc.tile_pool(name="wop", bufs=2) as wop, \
                 tc.tile_pool(name="ost", bufs=4) as ost, \
                 tc.tile_pool(name="ps_s", bufs=2, space="PSUM") as pss_, \
                 tc.tile_pool(name="ps_d", bufs=1, space="PSUM") as psd_, \
                 tc.tile_pool(name="ps_o", bufs=1, space="PSUM") as pso_, \
                 tc.tile_pool(name="ps_f", bufs=2, space="PSUM") as psf:

                attnT = [alp.tile([128, S], bf16, tag=f"attnT{h}",
                                  name=f"attnT{h}") for h in range(HL)]
                latT = [alp.tile([128, S], bf16, tag=f"latT{h}",
                                 name=f"latT{h}") for h in range(LHL)]

                # prefetch first o_proj weights; executes once SBUF frees
                wo2 = [None] * 4
                wlo2 = [None] * 4

                def load_np(np_):
                    wo2[np_] = wop.tile([128, HL, 1024], bf16, tag="wo2",
                                        name=f"wo2_{np_}")
                    wlo2[np_] = wop.tile([128, LHL, 1024], bf16, tag="wlo2",
                                         name=f"wlo2_{np_}")
                    for u in range(2):
                        nc.sync.dma_start(
                            wo2[np_][:, :, bass.ts(u, 512)],
                            wo_d.ap()[2 * np_ + u])
                        nc.sync.dma_start(
                            wlo2[np_][:, :, bass.ts(u, 512)],
                            wlo_d.ap()[2 * np_ + u])

                load_np(0)

                for vh in range(HL + LHL):
                    if vh < HL:
                        h = vh
                        ksrc = kT[h // GROUPS]
                        dst = attnT[h]
                        vof = (h // GROUPS) * HD
                        vtile = v_sb
                    else:
                        h = vh - HL
                        ksrc = lkT[h]
                        dst = latT[h]
                        vof = h * HD
                        vtile = lv_sb
                    qsrc = qT[h]

                    for ib in range(IB):
                        groups = _attn_blocks(ib)
                        pts = {}   # jb -> (pt_tile, off, width, sc)
                        for gi, (blocks, width) in enumerate(groups):
                            ps_s = pss_.tile([128, 1024], f32, tag="ps_s",
                                             name=f"ps_s_{vh}_{ib}_{gi}")
                            for (jb, off, w, sc) in blocks:
                                nc.tensor.matmul(
                                    ps_s[:, off:off + w],
                                    ksrc[:, bass.ts(jb, 128)],
                                    qsrc[:, ib * 512 + sc: (ib + 1) * 512],
                                    start=True, stop=True)
                            pt = ptp.tile([128, 1024], bf16, tag="pt",
                                          name=f"pt_{vh}_{ib}_{gi}")
                            nc.scalar.activation(
                                pt[:, 0:width], ps_s[:, 0:width],
                                mybir.ActivationFunctionType.Exp,
                                scale=SCALE)
                            for (jb, off, w, sc) in blocks:
                                if jb >= 4 * ib:   # diagonal: mask leading 128
                                    nc.vector.tensor_tensor(
                                        pt[:, off:off + 128],
                                        pt[:, off:off + 128], tri[:],
                                        mybir.AluOpType.mult)
                                pts[jb] = (pt, off, w, sc)

                        njb = 4 * (ib + 1)
                        ps_o = pso_.tile([128, 512], f32, tag="ps_o",
                                         name=f"ps_o_{vh}_{ib}")
                        for jb in range(njb):
                            pt, off, w, sc = pts[jb]
                            nc.tensor.matmul(
                                ps_o[:, sc:512],
                                vtile[:, jb, vof:vof + HD],
                                pt[:, off:off + w],
                                start=(jb == 0), stop=(jb == njb - 1),
                                skip_group_check=True)

                        denb = psd_.tile([128, 512], f32, tag="denb",
                                         name=f"denb_{vh}_{ib}")
                        for jb in range(njb):
                            pt, off, w, sc = pts[jb]
                            nc.tensor.matmul(
                                denb[:, sc:512], ones[:],
                                pt[:, off:off + w],
                                start=(jb == 0), stop=(jb == njb - 1),
                                skip_group_check=True)
                        rec = recp.tile([128, 512], f32, tag="rec",
                                        name=f"rec_{vh}_{ib}")
                        nc.vector.reciprocal_approx_fast(rec[:], denb[:])
                        nc.vector.tensor_tensor(
                            dst[:, bass.ts(ib, 512)], ps_o[:], rec[:],
                            mybir.AluOpType.mult)

                # ---- phase D: out += attn @ w_o + lat @ w_lo (row-shard) --
                for np_ in range(4):       # pairs of 512-wide col blocks
                    if np_ + 1 < 4:
                        load_np(np_ + 1)
                    for tt in range(S_T):
                        ps0 = psf.tile([128, 512], f32, tag="ps_f",
                                       name=f"ps_f0_{np_}_{tt}")
                        ps1 = psf.tile([128, 512], f32, tag="ps_f",
                                       name=f"ps_f1_{np_}_{tt}")
                        for ui, ps in enumerate((ps0, ps1)):
                            us = bass.ts(ui, 512)
                            for h in range(HL):
                                nc.tensor.matmul(
                                    ps[:], attnT[h][:, bass.ts(tt, 128)],
                                    wo2[np_][:, h, us],
                                    start=(h == 0), stop=False)
                            for h in range(LHL):
                                nc.tensor.matmul(
                                    ps[:], latT[h][:, bass.ts(tt, 128)],
                                    wlo2[np_][:, h, us],
                                    start=False, stop=(h == LHL - 1))
                        for u, ps in enumerate((ps0, ps1)):
                            ot = ost.tile([128, 512], f32, tag="ot",
                                          name=f"ot_{np_}_{tt}_{u}")
                            nc.any.tensor_copy(ot[:], ps[:])
                            nc.sync.dma_start(
                                out_ap[:, tt, bass.ds(
                                    (2 * np_ + u) * 512, 512)],
                                ot[:])

    nc.compile()
    return nc


_NC = None


def _get_program():
    global _NC
    if _NC is None:
        _NC = _build_program()
    return _NC


def _rope_tables():
    inv_freq = 1.0 / (THETA ** (np.arange(0, HD, 2, dtype=np.float32) / HD))
    t = np.arange(S, dtype=np.float32)
    freqs = np.outer(t, inv_freq)                       # [S, 64]
    emb = np.concatenate([freqs, freqs], axis=-1)       # [S, HD]
    cosT = np.cos(emb).T.astype(BF16).copy()            # [HD, S]
    sinT = np.sin(emb).T.astype(np.float32)
    sinTs = np.concatenate([-sinT[:HD // 2], sinT[HD // 2:]], 0).astype(
        BF16).copy()
    return cosT, sinTs


def _tri_mask():
    # tri[p, s] = 1.0 iff p <= s (k-token p attends-allowed for query col s)
    p = np.arange(128)[:, None]
    s = np.arange(128)[None, :]
    return (p <= s).astype(BF16)


def _tile_w_fm(w, n_tiles, kt):
    # [K, n_tiles*128] -> [n_tiles, 128(p of K), kt, 128]
    K, N = w.shape
    assert K == kt * 128 and N == n_tiles * 128
    return np.ascontiguousarray(
        w.reshape(kt, 128, n_tiles, 128).transpose(2, 1, 0, 3)).astype(BF16)


def _tile_w_tm(w, kt):
    # [K, N] -> [128(p of K), kt, N]
    K, N = w.shape
    assert K == kt * 128
    return np.ascontiguousarray(
        w.reshape(kt, 128, N).transpose(1, 0, 2)).astype(BF16)


def _tile_w_out(w):
    # [1024, D] -> [8(nb), 128(p of rows), 8(h), 512]
    return np.ascontiguousarray(
        w.reshape(8, 128, D // 512, 512).transpose(2, 1, 0, 3)).astype(BF16)


def _prep_core_inputs(inputs, c):
    """Build the per-core input map for core c from full fp32 inputs."""
    cosT, sinTs = _rope_tables()
    b, g = divmod(c, TP)
    x = np.asarray(inputs["hidden_states"][b], dtype=np.float32)   # [S, D]
    xt = np.ascontiguousarray(
        x.T.reshape(D_T, 128, S).transpose(1, 0, 2)).astype(BF16)
    qs = slice(g * HL * HD, (g + 1) * HL * HD)
    kvs = slice(g * KVL * HD, (g + 1) * KVL * HD)
    ls = slice(g * LHL * HD, (g + 1) * LHL * HD)
    wlkc = np.asarray(inputs["w_lq"], dtype=np.float32) @ np.asarray(
        inputs["w_lk"], dtype=np.float32)[:, ls]
    return {
        "xt": xt,
        "wq": _tile_w_fm(np.asarray(inputs["w_q"])[:, qs], HL, D_T),
        "wk": _tile_w_fm(np.asarray(inputs["w_k"])[:, kvs], KVL, D_T),
        "wv": _tile_w_tm(np.asarray(inputs["w_v"])[:, kvs], D_T),
        "wlkc": _tile_w_fm(wlkc, LHL, D_T),
        "wlv": _tile_w_tm(np.asarray(inputs["w_lv"])[:, ls], D_T),
        "wo": _tile_w_out(np.asarray(inputs["w_o"])[qs, :]),
        "wlo": _tile_w_out(np.asarray(inputs["w_lo"])[ls, :]),
        "cosT": cosT,
        "sinTs": sinTs,
        "trimask": _tri_mask(),
    }


def kernel(hidden_states, w_q, w_k, w_v, w_o, w_lq, w_lk, w_lv, w_lo):
    nc = _get_program()
    inputs = {
        "hidden_states": hidden_states, "w_q": w_q, "w_k": w_k, "w_v": w_v,
        "w_o": w_o, "w_lq": w_lq, "w_lk": w_lk, "w_lv": w_lv, "w_lo": w_lo,
    }
    in_maps = [_prep_core_inputs(inputs, c) for c in range(NCORES)]
    res = run_bass_kernel_spmd(nc, in_maps, list(range(NCORES))).results

    out = np.zeros((B, S, D), dtype=np.float32)
    for c in range(NCORES):
        out[c // TP] += res[c]["out"]
    return out


# revision 8
# speedup vs baseline: 1.2452x; 1.0030x over previous
"""Trainium2 Bass kernel for LlamaMultiheadLatentAttention.

Contract: kernel(**inputs) takes FULL fp32 inputs (as produced by
reference.setup_inputs) and returns the FULL fp32 output [2, 1024, 4096].

Sharding (8 cores, no collectives): core c handles batch b = c//4 and
head-group g = c%4 (8 query heads, 2 kv heads, 8 latent heads). q/k/v and
latent projections are column-sharded per head-group; o_proj/latent_o_proj
are row-sharded, so each core emits a partial output sum and the host adds
the 4 partials per batch.

Key layout/optimization choices:
  - lk is computed as x @ (w_lq @ w_lk) with the weight product folded on
    the host, removing the duplicated latent-q projection entirely.
  - activations feature-major (xT, qT, kT, lkT: [feat_p, tokens]) so every
    projection and attention matmul needs no transposes.
  - attention scores computed transposed, S^T[k, q] (k-tokens on partitions);
    causal structure exploited by trimming diagonal j-blocks to their valid
    query range and packing the trimmed blocks tightly into PSUM banks so a
    single exp instruction covers contiguous valid data.
  - softmax denominator via an all-ones [128,128] stationary matmul: the
    denominator arrives already broadcast across partitions in PSUM; the
    reciprocal uses the fast approximate DVE op (~18 bits, plenty here).
  - per-head SBUF tiles (qT_h, lkT_h, attnT_h, latT_h) give the Tile
    scheduler fine-grained dependencies, so projection, attention, and
    output-projection phases overlap instead of serializing.
  - all matmul operands bf16 (4x TensorE throughput vs fp32), fp32 PSUM.
"""

import numpy as np
import ml_dtypes

import concourse.bass as bass
import concourse.mybir as mybir
import concourse.tile as tile
from concourse import bacc
from concourse.bass_utils import run_bass_kernel_spmd

BF16 = ml_dtypes.bfloat16

B, S, D = 2, 1024, 4096
H, KVH, HD = 32, 8, 128
GROUPS = H // KVH
LAT, LH = 1024, 32
THETA = 10000.0
SCALE = 1.0 / float(np.sqrt(HD))

NCORES = 8
TP = 4                 # head-group shards
HL = H // TP           # 8 local q heads
KVL = KVH // TP        # 2 local kv heads
LHL = LH // TP         # 8 local latent heads

f32 = mybir.dt.float32
bf16 = mybir.dt.bfloat16

D_T = D // 128         # 32 k-tiles over model dim
S_T = S // 128         # 8 token tiles of 128
IB = 2                 # token blocks of 512


def _attn_blocks(ib):
    """Causal block layout for query block ib (512 queries).

    Returns list of (jb, off, width, sc) where jb is the key tile, off the
    column offset inside the score-group PSUM tile, width the number of valid
    query columns, and sc the query-column start within the 512-block.
    Grouped so that each group is one PSUM tile ([128, 1024] max, each
    matmul output within a single 512-col bank) and the valid columns are
    contiguous from 0 (one exp covers them with no gaps).
    """
    groups = []
    full = [jb for jb in range(4 * ib)]          # non-diagonal: full width
    for pair in range(len(full) // 2):
        a, b_ = full[2 * pair], full[2 * pair + 1]
        groups.append(([(a, 0, 512, 0), (b_, 512, 512, 0)], 1024))
    dg = 4 * ib
    # diagonal blocks dg+0..dg+3 with widths 512,384,256,128
    groups.append(([(dg, 0, 512, 0), (dg + 1, 512, 384, 128),
                    (dg + 3, 896, 128, 384)], 1024))
    groups.append(([(dg + 2, 0, 256, 256)], 256))
    return groups


def _build_program():
    nc = bacc.Bacc("TRN2", target_bir_lowering=False, debug=False)

    xt_d = nc.dram_tensor("xt", [128, D_T, S], bf16, kind="ExternalInput")
    wq_d = nc.dram_tensor("wq", [HL, 128, D_T, 128], bf16, kind="ExternalInput")
    wk_d = nc.dram_tensor("wk", [KVL, 128, D_T, 128], bf16, kind="ExternalInput")
    wv_d = nc.dram_tensor("wv", [128, D_T, KVL * HD], bf16, kind="ExternalInput")
    wlkc_d = nc.dram_tensor("wlkc", [LHL, 128, D_T, 128], bf16,
                            kind="ExternalInput")
    wlv_d = nc.dram_tensor("wlv", [128, D_T, LHL * HD], bf16,
                           kind="ExternalInput")
    wo_d = nc.dram_tensor("wo", [8, 128, HL, 512], bf16, kind="ExternalInput")
    wlo_d = nc.dram_tensor("wlo", [8, 128, LHL, 512], bf16,
                           kind="ExternalInput")
    cos_d = nc.dram_tensor("cosT", [HD, S], bf16, kind="ExternalInput")
    sin_d = nc.dram_tensor("sinTs", [HD, S], bf16, kind="ExternalInput")
    tri_d = nc.dram_tensor("trimask", [128, 128], bf16, kind="ExternalInput")
    out_d = nc.dram_tensor("out", [S, D], f32, kind="ExternalOutput")

    out_ap = out_d.ap().rearrange("(tt p) d -> p tt d", p=128)

    with tile.TileContext(nc) as tc:
        with tc.tile_pool(name="const", bufs=1) as constp, \
             tc.tile_pool(name="acts", bufs=1) as acts:

            cosT = constp.tile([HD, S], bf16, tag="cosT")
            sinTs = constp.tile([HD, S], bf16, tag="sinTs")
            tri = constp.tile([128, 128], bf16, tag="tri")
            ones = constp.tile([128, 128], bf16, tag="ones")
            nc.sync.dma_start(cosT[:], cos_d.ap())
            nc.sync.dma_start(sinTs[:], sin_d.ap())
            nc.sync.dma_start(tri[:], tri_d.ap())
            nc.vector.memset(ones[:], 1.0)

            # persistent activations (bf16), per-head tiles for fine deps
            v_sb = acts.tile([128, S_T, KVL * HD], bf16, tag="v")
            lv_sb = acts.tile([128, S_T, LHL * HD], bf16, tag="lv")
            kT = [acts.tile([128, S], bf16, tag=f"kT{i}", name=f"kT{i}")
                  for i in range(KVL)]
            qT = [acts.tile([128, S], bf16, tag=f"qT{h}", name=f"qT{h}")
                  for h in range(HL)]
            lkT = [acts.tile([128, S], bf16, tag=f"lkT{h}", name=f"lkT{h}")
                   for h in range(LHL)]
            with tc.tile_pool(name="xt", bufs=1) as xtp:
                xt = [xtp.tile([128, D_T // 4, S], bf16, tag=f"xt{c}", name=f"xt{c}")
                      for c in range(4)]
                for c in range(4):
                    nc.sync.dma_start(
                        xt[c][:], xt_d.ap()[:, bass.ts(c, D_T // 4), :])

                def xts(kt, sl):
                    return xt[kt // 8][:, kt % 8, sl]

                # ---- phase B1: feature-major projections q, k, lk (+rope) --
                with tc.tile_pool(name="wstr", bufs=3) as wstr, \
                     tc.tile_pool(name="rope", bufs=2) as ropep, \
                     tc.tile_pool(name="ps_b1", bufs=3, space="PSUM") as psb1:

                    def rope_to(dst, ps, ib):
                        sl = bass.ts(ib, 512)
                        rt = ropep.tile([128, 512], f32, tag="rt", name="rt")
                        qc = ropep.tile([128, 512], f32, tag="qc", name="qc")
                        nc.vector.tensor_tensor(
                            rt[0:64, :], ps[64:128, :], sinTs[0:64, sl],
                            mybir.AluOpType.mult)
                        nc.vector.tensor_tensor(
                            rt[64:128, :], ps[0:64, :], sinTs[64:128, sl],
                            mybir.AluOpType.mult)
                        nc.vector.tensor_tensor(
                            qc[:], ps[:], cosT[:, sl], mybir.AluOpType.mult)
                        nc.vector.tensor_add(dst, qc[:], rt[:])

                    def proj_head(w_dram, nt, dst):
                        # dst[:, :] = rope(w[nt].T @ xt)
                        wt = wstr.tile([128, D_T, 128], bf16, tag="w32",
                                       name=f"w_{w_dram.name}_{nt}")
                        nc.sync.dma_start(wt[:], w_dram.ap()[nt])
                        ps = [psb1.tile([128, 512], f32, tag="ps_b1",
                                        name=f"ps_{w_dram.name}_{nt}_{ib}")
                              for ib in range(IB)]
                        for kt in range(D_T):
                            for ib in range(IB):
                                nc.tensor.matmul(
                                    ps[ib][:], wt[:, kt, :],
                                    xts(kt, bass.ts(ib, 512)),
                                    start=(kt == 0), stop=(kt == D_T - 1))
                        for ib in range(IB):
                            rope_to(dst[:, bass.ts(ib, 512)], ps[ib][:], ib)

                    for i in range(KVL):
                        proj_head(wk_d, i, kT[i])
                    for h in range(HL):
                        proj_head(wq_d, h, qT[h])
                    for h in range(LHL):
                        proj_head(wlkc_d, h, lkT[h])

                # ---- phase B2: token-major projections v, lv ----
                with tc.tile_pool(name="wvp", bufs=1) as wvp, \
                     tc.tile_pool(name="wlvp", bufs=1) as wlvp, \
                     tc.tile_pool(name="ps_b2", bufs=2, space="PSUM") as psb2:
                    wv_sb = wvp.tile([128, D_T, KVL * HD], bf16, tag="wv")
                    for c in range(4):
                        nc.sync.dma_start(
                            wv_sb[:, bass.ts(c, D_T // 4), :],
                            wv_d.ap()[:, bass.ts(c, D_T // 4), :])
                    for half in range(2):
                        wlv_sb = wlvp.tile([128, D_T, 512], bf16, tag="wlvh",
                                          name=f"wlvh_{half}")
                        hs = bass.ts(half, 512)
                        for c in range(4):
                            nc.sync.dma_start(
                                wlv_sb[:, bass.ts(c, D_T // 4), :],
                                wlv_d.ap()[:, bass.ts(c, D_T // 4), hs])
                        for tp_ in range(S_T // 2):   # token-tile pairs
                            pss = []
                            for u in range(2):
                                pair = [psb2.tile(
                                    [128, 512], f32, tag="ps_lv",
                                    name=f"ps_lv_{half}_{tp_}_{u}")]
                                if half == 0:
                                    pair.append(psb2.tile(
                                        [128, KVL * HD], f32, tag="ps_v",
                                        name=f"ps_v_{tp_}_{u}"))
                                pss.append(pair)
                            for kt in range(D_T):
                                st = kt == 0
                                sp = kt == D_T - 1
                                for u in range(2):
                                    tt = 2 * tp_ + u
                                    lhs = xts(kt, bass.ts(tt, 128))
                                    nc.tensor.matmul(pss[u][0][:], lhs,
                                                     wlv_sb[:, kt, :],
                                                     start=st, stop=sp)
                                    if half == 0:
                                        nc.tensor.matmul(pss[u][1][:], lhs,
                                                         wv_sb[:, kt, :],
                                                         start=st, stop=sp)
                            for u in range(2):
                                tt = 2 * tp_ + u
                                nc.any.tensor_copy(lv_sb[:, tt, hs],
                                                   pss[u][0][:])
                                if half == 0:
                                    nc.any.tensor_copy(v_sb[:, tt, :],
                                                       pss[u][1][:])

            # ---- phase C: attention (16 virtual heads, causal-trimmed) ----
            # ---- phase D: output projections (overlapped via scheduler) ---
            with tc.tile_pool(name="attnlat", bufs=1) as alp, \
                 tc.tile_pool(name="pt", bufs=4) as ptp, \
                 tc.tile_pool(name="rec", bufs=2) as recp, \
                 tc.tile_pool(name="wop", bufs=2) as wop, \
                 tc.tile_pool(name="ost", bufs=4) as ost, \
                 tc.tile_pool(name="ps_s", bufs=2, space="PSUM") as pss_, \
                 tc.tile_pool(name="ps_o", bufs=2, space="PSUM") as pso_, \
                 tc.tile_pool(name="ps_f", bufs=2, space="PSUM") as psf:

                attnT = [alp.tile([128, S], bf16, tag=f"attnT{h}",
                                  name=f"attnT{h}") for h in range(HL)]
                latT = [alp.tile([128, S], bf16, tag=f"latT{h}",
                                 name=f"latT{h}") for h in range(LHL)]

                # prefetch first o_proj weights; executes once SBUF frees
                wo2 = [None] * 4
                wlo2 = [None] * 4

                def load_np(np_):
                    wo2[np_] = wop.tile([128, HL, 1024], bf16, tag="wo2",
                                        name=f"wo2_{np_}")
                    wlo2[np_] = wop.tile([128, LHL, 1024], bf16, tag="wlo2",
                                         name=f"wlo2_{np_}")
                    for u in range(2):
                        nc.sync.dma_start(
                            wo2[np_][:, :, bass.ts(u, 512)],
                            wo_d.ap()[2 * np_ + u])
                        nc.sync.dma_start(
                            wlo2[np_][:, :, bass.ts(u, 512)],
                            wlo_d.ap()[2 * np_ + u])

                load_np(0)

                for vh in range(HL + LHL):
                    if vh < HL:
                        h = vh
                        ksrc = kT[h // GROUPS]
                        dst = attnT[h]
                        vof = (h // GROUPS) * HD
                        vtile = v_sb
                    else:
                        h = vh - HL
                        ksrc = lkT[h]
                        dst = latT[h]
                        vof = h * HD
                        vtile = lv_sb
                    qsrc = qT[h]

                    for ib in range(IB):
                        groups = _attn_blocks(ib)
                        pts = {}   # jb -> (pt_tile, off, width, sc)
                        denb = None
                        for gi, (blocks, width) in enumerate(groups):
                            ps_s = pss_.tile([128, 1024], f32, tag="ps_s",
                                             name=f"ps_s_{vh}_{ib}_{gi}")
                            if gi == len(groups) - 1:
                                denb = ps_s   # bank1 unused by scores: holds
                                              # the softmax denominator
                            for (jb, off, w, sc) in blocks:
                                nc.tensor.matmul(
                                    ps_s[:, off:off + w],
                                    ksrc[:, bass.ts(jb, 128)],
                                    qsrc[:, ib * 512 + sc: (ib + 1) * 512],
                                    start=True, stop=True)
                            pt = ptp.tile([128, 1024], bf16, tag="pt",
                                          name=f"pt_{vh}_{ib}_{gi}")
                            nc.scalar.activation(
                                pt[:, 0:width], ps_s[:, 0:width],
                                mybir.ActivationFunctionType.Exp,
                                scale=SCALE)
                            for (jb, off, w, sc) in blocks:
                                if jb >= 4 * ib:   # diagonal: mask leading 128
                                    nc.vector.tensor_tensor(
                                        pt[:, off:off + 128],
                                        pt[:, off:off + 128], tri[:],
                                        mybir.AluOpType.mult)
                                pts[jb] = (pt, off, w, sc)

                        njb = 4 * (ib + 1)
                        ps_o = pso_.tile([128, 512], f32, tag="ps_o",
                                         name=f"ps_o_{vh}_{ib}")
                        for jb in range(njb):
                            pt, off, w, sc = pts[jb]
                            nc.tensor.matmul(
                                ps_o[:, sc:512],
                                vtile[:, jb, vof:vof + HD],
                                pt[:, off:off + w],
                                start=(jb == 0), stop=(jb == njb - 1),
                                skip_group_check=True)

                        for jb in range(njb):
                            pt, off, w, sc = pts[jb]
                            nc.tensor.matmul(
                                denb[:, 512 + sc:1024], ones[:],
                                pt[:, off:off + w],
                                start=(jb == 0), stop=(jb == njb - 1),
                                skip_group_check=True)
                        rec = recp.tile([128, 512], f32, tag="rec",
                                        name=f"rec_{vh}_{ib}")
                        nc.vector.reciprocal_approx_fast(rec[:],
                                                         denb[:, 512:1024])
                        nc.vector.tensor_tensor(
                            dst[:, bass.ts(ib, 512)], ps_o[:], rec[:],
                            mybir.AluOpType.mult)

                # ---- phase D: out += attn @ w_o + lat @ w_lo (row-shard) --
                for np_ in range(4):       # pairs of 512-wide col blocks
                    if np_ + 1 < 4:
                        load_np(np_ + 1)
                    for tt in range(S_T):
                        ps0 = psf.tile([128, 512], f32, tag="ps_f",
                                       name=f"ps_f0_{np_}_{tt}")
                        ps1 = psf.tile([128, 512], f32, tag="ps_f",
                                       name=f"ps_f1_{np_}_{tt}")
                        for ui, ps in enumerate((ps0, ps1)):
                            us = bass.ts(ui, 512)
                            for h in range(HL):
                                nc.tensor.matmul(
                                    ps[:], attnT[h][:, bass.ts(tt, 128)],
                                    wo2[np_][:, h, us],
                                    start=(h == 0), stop=False)
                            for h in range(LHL):
                                nc.tensor.matmul(
                                    ps[:], latT[h][:, bass.ts(tt, 128)],
                                    wlo2[np_][:, h, us],
                                    start=False, stop=(h == LHL - 1))
                        for u, ps in enumerate((ps0, ps1)):
                            ot = ost.tile([128, 512], f32, tag="ot",
                                          name=f"ot_{np_}_{tt}_{u}")
                            nc.any.tensor_copy(ot[:], ps[:])
                            nc.sync.dma_start(
                                out_ap[:, tt, bass.ds(
                                    (2 * np_ + u) * 512, 512)],
                                ot[:])

    nc.compile()
    return nc


_NC = None


def _get_program():
    global _NC
    if _NC is None:
        _NC = _build_program()
    return _NC


def _rope_tables():
    inv_freq = 1.0 / (THETA ** (np.arange(0, HD, 2, dtype=np.float32) / HD))
    t = np.arange(S, dtype=np.float32)
    freqs = np.outer(t, inv_freq)                       # [S, 64]
    emb = np.concatenate([freqs, freqs], axis=-1)       # [S, HD]
    cosT = np.cos(emb).T.astype(BF16).copy()            # [HD, S]
    sinT = np.sin(emb).T.astype(np.float32)
    sinTs = np.concatenate([-sinT[:HD // 2], sinT[HD // 2:]], 0).astype(
        BF16).copy()
    return cosT, sinTs


def _tri_mask():
    # tri[p, s] = 1.0 iff p <= s (k-token p attends-allowed for query col s)
    p = np.arange(128)[:, None]
    s = np.arange(128)[None, :]
    return (p <= s).astype(BF16)


def _tile_w_fm(w, n_tiles, kt):
    # [K, n_tiles*128] -> [n_tiles, 128(p of K), kt, 128]
    K, N = w.shape
    assert K == kt * 128 and N == n_tiles * 128
    return np.ascontiguousarray(
        w.reshape(kt, 128, n_tiles, 128).transpose(2, 1, 0, 3)).astype(BF16)


def _tile_w_tm(w, kt):
    # [K, N] -> [128(p of K), kt, N]
    K, N = w.shape
    assert K == kt * 128
    return np.ascontiguousarray(
        w.reshape(kt, 128, N).transpose(1, 0, 2)).astype(BF16)


def _tile_w_out(w):
    # [1024, D] -> [8(nb), 128(p of rows), 8(h), 512]
    return np.ascontiguousarray(
        w.reshape(8, 128, D // 512, 512).transpose(2, 1, 0, 3)).astype(BF16)


def _prep_core_inputs(inputs, c):
    """Build the per-core input map for core c from full fp32 inputs."""
    cosT, sinTs = _rope_tables()
    b, g = divmod(c, TP)
    x = np.asarray(inputs["hidden_states"][b], dtype=np.float32)   # [S, D]
    xt = np.ascontiguousarray(
        x.T.reshape(D_T, 128, S).transpose(1, 0, 2)).astype(BF16)
    qs = slice(g * HL * HD, (g + 1) * HL * HD)
    kvs = slice(g * KVL * HD, (g + 1) * KVL * HD)
    ls = slice(g * LHL * HD, (g + 1) * LHL * HD)
    wlkc = np.asarray(inputs["w_lq"], dtype=np.float32) @ np.asarray(
        inputs["w_lk"], dtype=np.float32)[:, ls]
    return {
        "xt": xt,
        "wq": _tile_w_fm(np.asarray(inputs["w_q"])[:, qs], HL, D_T),
        "wk": _tile_w_fm(np.asarray(inputs["w_k"])[:, kvs], KVL, D_T),
        "wv": _tile_w_tm(np.asarray(inputs["w_v"])[:, kvs], D_T),
        "wlkc": _tile_w_fm(wlkc, LHL, D_T),
        "wlv": _tile_w_tm(np.asarray(inputs["w_lv"])[:, ls], D_T),
        "wo": _tile_w_out(np.asarray(inputs["w_o"])[qs, :]),
        "wlo": _tile_w_out(np.asarray(inputs["w_lo"])[ls, :]),
        "cosT": cosT,
        "sinTs": sinTs,
        "trimask": _tri_mask(),
    }


def kernel(hidden_states, w_q, w_k, w_v, w_o, w_lq, w_lk, w_lv, w_lo):
    nc = _get_program()
    inputs = {
        "hidden_states": hidden_states, "w_q": w_q, "w_k": w_k, "w_v": w_v,
        "w_o": w_o, "w_lq": w_lq, "w_lk": w_lk, "w_lv": w_lv, "w_lo": w_lo,
    }
    in_maps = [_prep_core_inputs(inputs, c) for c in range(NCORES)]
    res = run_bass_kernel_spmd(nc, in_maps, list(range(NCORES))).results

    out = np.zeros((B, S, D), dtype=np.float32)
    for c in range(NCORES):
        out[c // TP] += res[c]["out"]
    return out


# revision 13
# speedup vs baseline: 1.2872x; 1.0338x over previous
"""Trainium2 Bass kernel for LlamaMultiheadLatentAttention.

Contract: kernel(**inputs) takes FULL fp32 inputs (as produced by
reference.setup_inputs) and returns the FULL fp32 output [2, 1024, 4096].

Sharding (8 cores, no collectives): core c handles batch b = c//4 and
head-group g = c%4 (8 query heads, 2 kv heads, 8 latent heads). q/k/v and
latent projections are column-sharded per head-group; o_proj/latent_o_proj
are row-sharded, so each core emits a partial output sum and the host adds
the 4 partials per batch.

Key layout/optimization choices:
  - lk is computed as x @ (w_lq @ w_lk) with the weight product folded on
    the host, removing the duplicated latent-q projection entirely.
  - activations feature-major (xT, qT, kT, lkT: [feat_p, tokens]) so every
    projection and attention matmul needs no transposes.
  - attention scores computed transposed, S^T[k, q] (k-tokens on partitions);
    causal structure exploited by trimming diagonal j-blocks to their valid
    query range and packing the trimmed blocks tightly into PSUM banks so a
    single exp instruction covers contiguous valid data.
  - softmax denominator via an all-ones [128,128] stationary matmul (arrives
    broadcast across partitions, in the unused second bank of the last score
    group's PSUM tile); reciprocal via the fast approximate DVE op.
  - per-head SBUF tiles (qT_h, lkT_h, attnT_h, latT_h) give the Tile
    scheduler fine-grained dependencies, so projection, attention, and
    output-projection phases overlap instead of serializing.
  - first four projections run kt-major as a group so TensorE saturates
    while the xt chunks stream in from HBM.
  - all matmul operands bf16 (fp8 was measured: weight-quantization error is
    token-correlated and passes undamped through softmax -> 3.6e-2 rel err,
    over the gate), fp32 PSUM.
"""

import numpy as np
import ml_dtypes

import concourse.bass as bass
import concourse.mybir as mybir
import concourse.tile as tile
from concourse import bacc
from concourse.bass_utils import run_bass_kernel_spmd

BF16 = ml_dtypes.bfloat16

B, S, D = 2, 1024, 4096
H, KVH, HD = 32, 8, 128
GROUPS = H // KVH
LAT, LH = 1024, 32
THETA = 10000.0
SCALE = 1.0 / float(np.sqrt(HD))

NCORES = 8
TP = 4                 # head-group shards
HL = H // TP           # 8 local q heads
KVL = KVH // TP        # 2 local kv heads
LHL = LH // TP         # 8 local latent heads

f32 = mybir.dt.float32
bf16 = mybir.dt.bfloat16

D_T = D // 128         # 32 k-tiles over model dim
S_T = S // 128         # 8 token tiles of 128
IB = 2                 # token blocks of 512


def _attn_blocks(ib):
    """Causal block layout for query block ib (512 queries).

    Returns groups of (jb, off, width, sc): jb key tile, off column offset in
    the score-group PSUM tile, width valid query columns, sc query-column
    start within the 512-block. Each group is one [128, 1024] PSUM tile (each
    matmul output within a single 512-col bank) with valid columns contiguous
    from 0 so one exp covers them with no gaps.
    """
    groups = []
    full = list(range(4 * ib))                   # non-diagonal: full width
    for pair in range(len(full) // 2):
        a, b_ = full[2 * pair], full[2 * pair + 1]
        groups.append(([(a, 0, 512, 0), (b_, 512, 512, 0)], 1024))
    dg = 4 * ib
    # diagonal blocks dg+0..dg+3 with widths 512,384,256,128
    groups.append(([(dg, 0, 512, 0), (dg + 1, 512, 384, 128),
                    (dg + 3, 896, 128, 384)], 1024))
    groups.append(([(dg + 2, 0, 256, 256)], 256))
    return groups


def _build_program():
    nc = bacc.Bacc("TRN2", target_bir_lowering=False, debug=False)

    xt_d = nc.dram_tensor("xt", [128, D_T, S], bf16, kind="ExternalInput")
    wq_d = nc.dram_tensor("wq", [HL, 128, D_T, 128], bf16, kind="ExternalInput")
    wk_d = nc.dram_tensor("wk", [KVL, 128, D_T, 128], bf16, kind="ExternalInput")
    wv_d = nc.dram_tensor("wv", [128, D_T, KVL * HD], bf16, kind="ExternalInput")
    wlkc_d = nc.dram_tensor("wlkc", [LHL, 128, D_T, 128], bf16,
                            kind="ExternalInput")
    wlv_d = nc.dram_tensor("wlv", [128, D_T, LHL * HD], bf16,
                           kind="ExternalInput")
    wo_d = nc.dram_tensor("wo", [8, 128, HL, 512], bf16, kind="ExternalInput")
    wlo_d = nc.dram_tensor("wlo", [8, 128, LHL, 512], bf16,
                           kind="ExternalInput")
    cos_d = nc.dram_tensor("cosT", [HD, S], bf16, kind="ExternalInput")
    sin_d = nc.dram_tensor("sinTs", [HD, S], bf16, kind="ExternalInput")
    tri_d = nc.dram_tensor("trimask", [128, 128], bf16, kind="ExternalInput")
    out_d = nc.dram_tensor("out", [S, D], f32, kind="ExternalOutput")

    out_ap = out_d.ap().rearrange("(tt p) d -> p tt d", p=128)

    with tile.TileContext(nc) as tc:
        with tc.tile_pool(name="const", bufs=1) as constp, \
             tc.tile_pool(name="acts", bufs=1) as acts:

            cosT = constp.tile([HD, S], bf16, tag="cosT")
            sinTs = constp.tile([HD, S], bf16, tag="sinTs")
            tri = constp.tile([128, 128], bf16, tag="tri")
            ones = constp.tile([128, 128], bf16, tag="ones")
            nc.vector.memset(ones[:], 1.0)

            # persistent activations (bf16), per-head tiles for fine deps
            v_sb = acts.tile([128, S_T, KVL * HD], bf16, tag="v")
            lv_sb = acts.tile([128, S_T, LHL * HD], bf16, tag="lv")
            kT = [acts.tile([128, S], bf16, tag=f"kT{i}", name=f"kT{i}")
                  for i in range(KVL)]
            qT = [acts.tile([128, S], bf16, tag=f"qT{h}", name=f"qT{h}")
                  for h in range(HL)]
            lkT = [acts.tile([128, S], bf16, tag=f"lkT{h}", name=f"lkT{h}")
                   for h in range(LHL)]

            with tc.tile_pool(name="xt", bufs=1) as xtp:
                xt = [xtp.tile([128, D_T // 4, S], bf16, tag=f"xt{c}",
                               name=f"xt{c}") for c in range(4)]
                nc.sync.dma_start(xt[0][:], xt_d.ap()[:, bass.ts(0, 8), :])

                def xts(kt, sl):
                    return xt[kt // 8][:, kt % 8, sl]

                # ---- phase B1: feature-major projections q, k, lk (+rope) --
                with tc.tile_pool(name="wstr", bufs=4) as wstr, \
                     tc.tile_pool(name="rope", bufs=2) as ropep, \
                     tc.tile_pool(name="ps_b1", bufs=8, space="PSUM") as psb1:

                    def rope_to(dst, ps, ib, nm):
                        sl = bass.ts(ib, 512)
                        rt = ropep.tile([128, 512], f32, tag="rt",
                                        name=f"rt_{nm}")
                        qc = ropep.tile([128, 512], f32, tag="qc",
                                        name=f"qc_{nm}")
                        nc.vector.tensor_tensor(
                            rt[0:64, :], ps[64:128, :], sinTs[0:64, sl],
                            mybir.AluOpType.mult)
                        nc.vector.tensor_tensor(
                            rt[64:128, :], ps[0:64, :], sinTs[64:128, sl],
                            mybir.AluOpType.mult)
                        nc.vector.tensor_tensor(
                            qc[:], ps[:], cosT[:, sl], mybir.AluOpType.mult)
                        nc.vector.tensor_add(dst, qc[:], rt[:])

                    # -- startup group: k0, k1, q0, q1 kt-major so TensorE
                    #    saturates while xt chunks stream in --
                    g4 = [(wk_d, 0, kT[0]), (wk_d, 1, kT[1]),
                          (wq_d, 0, qT[0]), (wq_d, 1, qT[1])]
                    wts, pss4 = [], []
                    for w_dram, nt, dst in g4:
                        wt = wstr.tile([128, D_T, 128], bf16, tag="w32",
                                       name=f"w_{w_dram.name}_{nt}")
                        nc.sync.dma_start(wt[:], w_dram.ap()[nt])
                        wts.append(wt)
                        pss4.append([psb1.tile(
                            [128, 512], f32, tag="ps_b1",
                            name=f"ps_{w_dram.name}_{nt}_{ib}")
                            for ib in range(IB)])
                    for c in range(1, 4):
                        nc.sync.dma_start(
                            xt[c][:], xt_d.ap()[:, bass.ts(c, 8), :])
                    nc.sync.dma_start(cosT[:], cos_d.ap())
                    nc.sync.dma_start(sinTs[:], sin_d.ap())
                    nc.sync.dma_start(tri[:], tri_d.ap())
                    for kt in range(D_T):
                        for gi in range(4):
                            for ib in range(IB):
                                nc.tensor.matmul(
                                    pss4[gi][ib][:], wts[gi][:, kt, :],
                                    xts(kt, bass.ts(ib, 512)),
                                    start=(kt == 0), stop=(kt == D_T - 1))
                    for gi, (w_dram, nt, dst) in enumerate(g4):
                        for ib in range(IB):
                            rope_to(dst[:, bass.ts(ib, 512)], pss4[gi][ib][:],
                                    ib, f"{w_dram.name}{nt}_{ib}")

                    def proj_head(w_dram, nt, dst):
                        # dst[:, :] = rope(w[nt].T @ xt)
                        wt = wstr.tile([128, D_T, 128], bf16, tag="w32",
                                       name=f"w_{w_dram.name}_{nt}")
                        nc.sync.dma_start(wt[:], w_dram.ap()[nt])
                        ps = [psb1.tile([128, 512], f32, tag="ps_b1",
                                        name=f"ps_{w_dram.name}_{nt}_{ib}")
                              for ib in range(IB)]
                        for kt in range(D_T):
                            for ib in range(IB):
                                nc.tensor.matmul(
                                    ps[ib][:], wt[:, kt, :],
                                    xts(kt, bass.ts(ib, 512)),
                                    start=(kt == 0), stop=(kt == D_T - 1))
                        for ib in range(IB):
                            rope_to(dst[:, bass.ts(ib, 512)], ps[ib][:], ib,
                                    f"{w_dram.name}{nt}_{ib}")

                    for h in range(2, HL):
                        proj_head(wq_d, h, qT[h])
                    for h in range(LHL):
                        proj_head(wlkc_d, h, lkT[h])

                # ---- phase B2: token-major projections v, lv ----
                with tc.tile_pool(name="wvp", bufs=1) as wvp, \
                     tc.tile_pool(name="wlvp", bufs=2) as wlvp, \
                     tc.tile_pool(name="ps_b2", bufs=2, space="PSUM") as psb2:
                    wv_sb = wvp.tile([128, D_T, KVL * HD], bf16, tag="wv")
                    for c in range(4):
                        nc.sync.dma_start(
                            wv_sb[:, bass.ts(c, D_T // 4), :],
                            wv_d.ap()[:, bass.ts(c, D_T // 4), :])
                    for half in range(2):
                        wlv_sb = wlvp.tile([128, D_T, 512], bf16, tag="wlvh",
                                           name=f"wlvh_{half}")
                        hs = bass.ts(half, 512)
                        for c in range(4):
                            nc.sync.dma_start(
                                wlv_sb[:, bass.ts(c, D_T // 4), :],
                                wlv_d.ap()[:, bass.ts(c, D_T // 4), hs])
                        for tp_ in range(S_T // 2):   # token-tile pairs
                            pss = []
                            for u in range(2):
                                pair = [psb2.tile(
                                    [128, 512], f32, tag="ps_lv",
                                    name=f"ps_lv_{half}_{tp_}_{u}")]
                                if half == 0:
                                    pair.append(psb2.tile(
                                        [128, KVL * HD], f32, tag="ps_v",
                                        name=f"ps_v_{tp_}_{u}"))
                                pss.append(pair)
                            for kt in range(D_T):
                                st = kt == 0
                                sp = kt == D_T - 1
                                for u in range(2):
                                    tt = 2 * tp_ + u
                                    lhs = xts(kt, bass.ts(tt, 128))
                                    nc.tensor.matmul(pss[u][0][:], lhs,
                                                     wlv_sb[:, kt, :],
                                                     start=st, stop=sp)
                                    if half == 0:
                                        nc.tensor.matmul(pss[u][1][:], lhs,
                                                         wv_sb[:, kt, :],
                                                         start=st, stop=sp)
                            for u in range(2):
                                tt = 2 * tp_ + u
                                nc.any.tensor_copy(lv_sb[:, tt, hs],
                                                   pss[u][0][:])
                                if half == 0:
                                    nc.any.tensor_copy(v_sb[:, tt, :],
                                                       pss[u][1][:])

            # ---- phase C: attention (16 virtual heads, causal-trimmed) ----
            # ---- phase D: output projections (overlapped via scheduler) ---
            with tc.tile_pool(name="attnlat", bufs=1) as alp, \
                 tc.tile_pool(name="pt", bufs=4) as ptp, \
                 tc.tile_pool(name="rec", bufs=2) as recp, \
                 tc.tile_pool(name="wop", bufs=2) as wop, \
                 tc.tile_pool(name="ost", bufs=4) as ost, \
                 tc.tile_pool(name="ps_s", bufs=2, space="PSUM") as pss_, \
                 tc.tile_pool(name="ps_o", bufs=2, space="PSUM") as pso_, \
                 tc.tile_pool(name="ps_f", bufs=2, space="PSUM") as psf:

                attnT = [alp.tile([128, S], bf16, tag=f"attnT{h}",
                                  name=f"attnT{h}") for h in range(HL)]
                latT = [alp.tile([128, S], bf16, tag=f"latT{h}",
                                 name=f"latT{h}") for h in range(LHL)]

                # prefetch first o_proj weights; executes once SBUF frees
                wo2 = [None] * 4
                wlo2 = [None] * 4

                def load_np(np_):
                    wo2[np_] = wop.tile([128, HL, 1024], bf16, tag="wo2",
                                        name=f"wo2_{np_}")
                    wlo2[np_] = wop.tile([128, LHL, 1024], bf16, tag="wlo2",
                                         name=f"wlo2_{np_}")
                    for u in range(2):
                        nc.sync.dma_start(
                            wo2[np_][:, :, bass.ts(u, 512)],
                            wo_d.ap()[2 * np_ + u])
                        nc.sync.dma_start(
                            wlo2[np_][:, :, bass.ts(u, 512)],
                            wlo_d.ap()[2 * np_ + u])

                load_np(0)

                for vh in range(HL + LHL):
                    if vh < HL:
                        h = vh
                        ksrc = kT[h // GROUPS]
                        dst = attnT[h]
                        vof = (h // GROUPS) * HD
                        vtile = v_sb
                    else:
                        h = vh - HL
                        ksrc = lkT[h]
                        dst = latT[h]
                        vof = h * HD
                        vtile = lv_sb
                    qsrc = qT[h]

                    for ib in range(IB):
                        groups = _attn_blocks(ib)
                        pts = {}   # jb -> (pt_tile, off, width, sc)
                        denb = None
                        for gi, (blocks, width) in enumerate(groups):
                            ps_s = pss_.tile([128, 1024], f32, tag="ps_s",
                                             name=f"ps_s_{vh}_{ib}_{gi}")
                            if gi == len(groups) - 1:
                                denb = ps_s   # bank1 unused by scores: holds
                                              # the softmax denominator
                            for (jb, off, w, sc) in blocks:
                                nc.tensor.matmul(
                                    ps_s[:, off:off + w],
                                    ksrc[:, bass.ts(jb, 128)],
                                    qsrc[:, ib * 512 + sc: (ib + 1) * 512],
                                    start=True, stop=True)
                            pt = ptp.tile([128, 1024], bf16, tag="pt",
                                          name=f"pt_{vh}_{ib}_{gi}")
                            nc.scalar.activation(
                                pt[:, 0:width], ps_s[:, 0:width],
                                mybir.ActivationFunctionType.Exp,
                                scale=SCALE)
                            for (jb, off, w, sc) in blocks:
                                if jb >= 4 * ib:   # diagonal: mask leading 128
                                    nc.vector.tensor_tensor(
                                        pt[:, off:off + 128],
                                        pt[:, off:off + 128], tri[:],
                                        mybir.AluOpType.mult)
                                pts[jb] = (pt, off, w, sc)

                        njb = 4 * (ib + 1)
                        for jb in range(njb):
                            pt, off, w, sc = pts[jb]
                            nc.tensor.matmul(
                                denb[:, 512 + sc:1024], ones[:],
                                pt[:, off:off + w],
                                start=(jb == 0), stop=(jb == njb - 1),
                                skip_group_check=True)
                        rec = recp.tile([128, 512], f32, tag="rec",
                                        name=f"rec_{vh}_{ib}")
                        nc.vector.reciprocal_approx_fast(rec[:],
                                                         denb[:, 512:1024])
                        ps_o = pso_.tile([128, 512], f32, tag="ps_o",
                                         name=f"ps_o_{vh}_{ib}")
                        for jb in range(njb):
                            pt, off, w, sc = pts[jb]
                            nc.tensor.matmul(
                                ps_o[:, sc:512],
                                vtile[:, jb, vof:vof + HD],
                                pt[:, off:off + w],
                                start=(jb == 0), stop=(jb == njb - 1),
                                skip_group_check=True)
                        nc.vector.tensor_tensor(
                            dst[:, bass.ts(ib, 512)], ps_o[:], rec[:],
                            mybir.AluOpType.mult)

                # ---- phase D: out += attn @ w_o + lat @ w_lo (row-shard) --
                for np_ in range(4):       # pairs of 512-wide col blocks
                    if np_ + 1 < 4:
                        load_np(np_ + 1)
                    for tt in range(S_T):
                        ps0 = psf.tile([128, 512], f32, tag="ps_f",
                                       name=f"ps_f0_{np_}_{tt}")
                        ps1 = psf.tile([128, 512], f32, tag="ps_f",
                                       name=f"ps_f1_{np_}_{tt}")
                        for ui, ps in enumerate((ps0, ps1)):
                            us = bass.ts(ui, 512)
                            for h in range(HL):
                                nc.tensor.matmul(
                                    ps[:], attnT[h][:, bass.ts(tt, 128)],
                                    wo2[np_][:, h, us],
                                    start=(h == 0), stop=False)
                            for h in range(LHL):
                                nc.tensor.matmul(
                                    ps[:], latT[h][:, bass.ts(tt, 128)],
                                    wlo2[np_][:, h, us],
                                    start=False, stop=(h == LHL - 1))
                        for u, ps in enumerate((ps0, ps1)):
                            ot = ost.tile([128, 512], f32, tag="ot",
                                          name=f"ot_{np_}_{tt}_{u}")
                            nc.any.tensor_copy(ot[:], ps[:])
                            nc.sync.dma_start(
                                out_ap[:, tt, bass.ds(
                                    (2 * np_ + u) * 512, 512)],
                                ot[:])

    nc.compile()
    return nc


_NC = None


def _get_program():
    global _NC
    if _NC is None:
        _NC = _build_program()
    return _NC


def _rope_tables():
    inv_freq = 1.0 / (THETA ** (np.arange(0, HD, 2, dtype=np.float32) / HD))
    t = np.arange(S, dtype=np.float32)
    freqs = np.outer(t, inv_freq)                       # [S, 64]
    emb = np.concatenate([freqs, freqs], axis=-1)       # [S, HD]
    cosT = np.cos(emb).T.astype(BF16).copy()            # [HD, S]
    sinT = np.sin(emb).T.astype(np.float32)
    sinTs = np.concatenate([-sinT[:HD // 2], sinT[HD // 2:]], 0).astype(
        BF16).copy()
    return cosT, sinTs


def _tri_mask():
    # tri[p, s] = 1.0 iff p <= s (k-token p attends-allowed for query col s)
    p = np.arange(128)[:, None]
    s = np.arange(128)[None, :]
    return (p <= s).astype(BF16)


def _tile_w_fm(w, n_tiles, kt):
    # [K, n_tiles*128] -> [n_tiles, 128(p of K), kt, 128]
    K, N = w.shape
    assert K == kt * 128 and N == n_tiles * 128
    return np.ascontiguousarray(
        np.asarray(w, dtype=np.float32).reshape(
            kt, 128, n_tiles, 128).transpose(2, 1, 0, 3)).astype(BF16)


def _tile_w_tm(w, kt):
    # [K, N] -> [128(p of K), kt, N]
    K, N = w.shape
    assert K == kt * 128
    return np.ascontiguousarray(
        np.asarray(w, dtype=np.float32).reshape(
            kt, 128, N).transpose(1, 0, 2)).astype(BF16)


def _tile_w_out(w):
    # [1024, D] -> [8(nb), 128(p of rows), 8(h), 512]
    return np.ascontiguousarray(
        w.reshape(8, 128, D // 512, 512).transpose(2, 1, 0, 3)).astype(BF16)


def _prep_core_inputs(inputs, c):
    """Build the per-core input map for core c from full fp32 inputs."""
    cosT, sinTs = _rope_tables()
    b, g = divmod(c, TP)
    x = np.asarray(inputs["hidden_states"][b], dtype=np.float32)   # [S, D]
    xt = np.ascontiguousarray(
        x.T.reshape(D_T, 128, S).transpose(1, 0, 2)).astype(BF16)
    qs = slice(g * HL * HD, (g + 1) * HL * HD)
    kvs = slice(g * KVL * HD, (g + 1) * KVL * HD)
    ls = slice(g * LHL * HD, (g + 1) * LHL * HD)
    wlkc = np.asarray(inputs["w_lq"], dtype=np.float32) @ np.asarray(
        inputs["w_lk"], dtype=np.float32)[:, ls]
    return {
        "xt": xt,
        "wq": _tile_w_fm(np.asarray(inputs["w_q"])[:, qs], HL, D_T),
        "wk": _tile_w_fm(np.asarray(inputs["w_k"])[:, kvs], KVL, D_T),
        "wv": _tile_w_tm(np.asarray(inputs["w_v"])[:, kvs], D_T),
        "wlkc": _tile_w_fm(wlkc, LHL, D_T),
        "wlv": _tile_w_tm(np.asarray(inputs["w_lv"])[:, ls], D_T),
        "wo": _tile_w_out(np.asarray(inputs["w_o"])[qs, :]),
        "wlo": _tile_w_out(np.asarray(inputs["w_lo"])[ls, :]),
        "cosT": cosT,
        "sinTs": sinTs,
        "trimask": _tri_mask(),
    }


def kernel(hidden_states, w_q, w_k, w_v, w_o, w_lq, w_lk, w_lv, w_lo):
    nc = _get_program()
    inputs = {
        "hidden_states": hidden_states, "w_q": w_q, "w_k": w_k, "w_v": w_v,
        "w_o": w_o, "w_lq": w_lq, "w_lk": w_lk, "w_lv": w_lv, "w_lo": w_lo,
    }
    in_maps = [_prep_core_inputs(inputs, c) for c in range(NCORES)]
    res = run_bass_kernel_spmd(nc, in_maps, list(range(NCORES))).results

    out = np.zeros((B, S, D), dtype=np.float32)
    for c in range(NCORES):
        out[c // TP] += res[c]["out"]
    return out


# revision 14
# speedup vs baseline: 1.2896x; 1.0019x over previous
"""Trainium2 Bass kernel for LlamaMultiheadLatentAttention.

Contract: kernel(**inputs) takes FULL fp32 inputs (as produced by
reference.setup_inputs) and returns the FULL fp32 output [2, 1024, 4096].

Sharding (8 cores, no collectives): core c handles batch b = c//4 and
head-group g = c%4 (8 query heads, 2 kv heads, 8 latent heads). q/k/v and
latent projections are column-sharded per head-group; o_proj/latent_o_proj
are row-sharded, so each core emits a partial output sum and the host adds
the 4 partials per batch.

Key layout/optimization choices:
  - lk is computed as x @ (w_lq @ w_lk) with the weight product folded on
    the host, removing the duplicated latent-q projection entirely.
  - activations feature-major (xT, qT, kT, lkT: [feat_p, tokens]) so every
    projection and attention matmul needs no transposes.
  - attention scores computed transposed, S^T[k, q] (k-tokens on partitions);
    causal structure exploited by trimming diagonal j-blocks to their valid
    query range and packing the trimmed blocks tightly into PSUM banks so a
    single exp instruction covers contiguous valid data.
  - softmax denominator via an all-ones [128,128] stationary matmul (arrives
    broadcast across partitions, in the unused second bank of the last score
    group's PSUM tile); reciprocal via the fast approximate DVE op.
  - per-head SBUF tiles (qT_h, lkT_h, attnT_h, latT_h) give the Tile
    scheduler fine-grained dependencies, so projection, attention, and
    output-projection phases overlap instead of serializing.
  - first four projections run kt-major as a group so TensorE saturates
    while the xt chunks stream in from HBM.
  - all matmul operands bf16 (fp8 was measured: weight-quantization error is
    token-correlated and passes undamped through softmax -> 3.6e-2 rel err,
    over the gate), fp32 PSUM.
"""

import numpy as np
import ml_dtypes

import concourse.bass as bass
import concourse.mybir as mybir
import concourse.tile as tile
from concourse import bacc
from concourse.bass_utils import run_bass_kernel_spmd

BF16 = ml_dtypes.bfloat16

B, S, D = 2, 1024, 4096
H, KVH, HD = 32, 8, 128
GROUPS = H // KVH
LAT, LH = 1024, 32
THETA = 10000.0
SCALE = 1.0 / float(np.sqrt(HD))

NCORES = 8
TP = 4                 # head-group shards
HL = H // TP           # 8 local q heads
KVL = KVH // TP        # 2 local kv heads
LHL = LH // TP         # 8 local latent heads

f32 = mybir.dt.float32
bf16 = mybir.dt.bfloat16

D_T = D // 128         # 32 k-tiles over model dim
S_T = S // 128         # 8 token tiles of 128
IB = 2                 # token blocks of 512


def _attn_blocks(ib):
    """Causal block layout for query block ib (512 queries).

    Returns groups of (jb, off, width, sc): jb key tile, off column offset in
    the score-group PSUM tile, width valid query columns, sc query-column
    start within the 512-block. Each group is one [128, 1024] PSUM tile (each
    matmul output within a single 512-col bank) with valid columns contiguous
    from 0 so one exp covers them with no gaps.
    """
    groups = []
    full = list(range(4 * ib))                   # non-diagonal: full width
    for pair in range(len(full) // 2):
        a, b_ = full[2 * pair], full[2 * pair + 1]
        groups.append(([(a, 0, 512, 0), (b_, 512, 512, 0)], 1024))
    dg = 4 * ib
    # diagonal blocks dg+0..dg+3 with widths 512,384,256,128
    groups.append(([(dg, 0, 512, 0), (dg + 1, 512, 384, 128),
                    (dg + 3, 896, 128, 384)], 1024))
    groups.append(([(dg + 2, 0, 256, 256)], 256))
    return groups


def _build_program():
    nc = bacc.Bacc("TRN2", target_bir_lowering=False, debug=False)

    xt_d = nc.dram_tensor("xt", [128, D_T, S], bf16, kind="ExternalInput")
    wq_d = nc.dram_tensor("wq", [HL, 128, D_T, 128], bf16, kind="ExternalInput")
    wk_d = nc.dram_tensor("wk", [KVL, 128, D_T, 128], bf16, kind="ExternalInput")
    wv_d = nc.dram_tensor("wv", [128, D_T, KVL * HD], bf16, kind="ExternalInput")
    wlkc_d = nc.dram_tensor("wlkc", [LHL, 128, D_T, 128], bf16,
                            kind="ExternalInput")
    wlv_d = nc.dram_tensor("wlv", [128, D_T, LHL * HD], bf16,
                           kind="ExternalInput")
    wo_d = nc.dram_tensor("wo", [8, 128, HL, 512], bf16, kind="ExternalInput")
    wlo_d = nc.dram_tensor("wlo", [8, 128, LHL, 512], bf16,
                           kind="ExternalInput")
    cos_d = nc.dram_tensor("cosT", [HD, S], bf16, kind="ExternalInput")
    sin_d = nc.dram_tensor("sinTs", [HD, S], bf16, kind="ExternalInput")
    tri_d = nc.dram_tensor("trimask", [128, 128], bf16, kind="ExternalInput")
    out_d = nc.dram_tensor("out", [S, D], f32, kind="ExternalOutput")

    out_ap = out_d.ap().rearrange("(tt p) d -> p tt d", p=128)

    with tile.TileContext(nc) as tc:
        with tc.tile_pool(name="const", bufs=1) as constp, \
             tc.tile_pool(name="acts", bufs=1) as acts:

            cosT = constp.tile([HD, S], bf16, tag="cosT")
            sinTs = constp.tile([HD, S], bf16, tag="sinTs")
            tri = constp.tile([128, 128], bf16, tag="tri")
            ones = constp.tile([128, 128], bf16, tag="ones")
            nc.vector.memset(ones[:], 1.0)

            # persistent activations (bf16), per-head tiles for fine deps
            v_sb = acts.tile([128, S_T, KVL * HD], bf16, tag="v")
            lv_sb = acts.tile([128, S_T, LHL * HD], bf16, tag="lv")
            kT = [acts.tile([128, S], bf16, tag=f"kT{i}", name=f"kT{i}")
                  for i in range(KVL)]
            qT = [acts.tile([128, S], bf16, tag=f"qT{h}", name=f"qT{h}")
                  for h in range(HL)]
            lkT = [acts.tile([128, S], bf16, tag=f"lkT{h}", name=f"lkT{h}")
                   for h in range(LHL)]

            with tc.tile_pool(name="xt", bufs=1) as xtp:
                xt = [xtp.tile([128, D_T // 8, S], bf16, tag=f"xt{c}",
                               name=f"xt{c}") for c in range(8)]
                nc.sync.dma_start(xt[0][:], xt_d.ap()[:, bass.ts(0, 4), :])

                def xts(kt, sl):
                    return xt[kt // 4][:, kt % 4, sl]

                # ---- phase B1: feature-major projections q, k, lk (+rope) --
                with tc.tile_pool(name="wstr", bufs=4) as wstr, \
                     tc.tile_pool(name="rope", bufs=2) as ropep, \
                     tc.tile_pool(name="ps_b1", bufs=8, space="PSUM") as psb1:

                    def rope_to(dst, ps, ib, nm):
                        sl = bass.ts(ib, 512)
                        rt = ropep.tile([128, 512], f32, tag="rt",
                                        name=f"rt_{nm}")
                        qc = ropep.tile([128, 512], f32, tag="qc",
                                        name=f"qc_{nm}")
                        nc.vector.tensor_tensor(
                            rt[0:64, :], ps[64:128, :], sinTs[0:64, sl],
                            mybir.AluOpType.mult)
                        nc.vector.tensor_tensor(
                            rt[64:128, :], ps[0:64, :], sinTs[64:128, sl],
                            mybir.AluOpType.mult)
                        nc.vector.tensor_tensor(
                            qc[:], ps[:], cosT[:, sl], mybir.AluOpType.mult)
                        nc.vector.tensor_add(dst, qc[:], rt[:])

                    # -- startup group: k0, k1, q0, q1 kt-major so TensorE
                    #    saturates while xt chunks stream in --
                    g4 = [(wk_d, 0, kT[0]), (wk_d, 1, kT[1]),
                          (wq_d, 0, qT[0])]
                    wts, pss4 = [], []
                    for w_dram, nt, dst in g4:
                        wt = wstr.tile([128, D_T, 128], bf16, tag="w32",
                                       name=f"w_{w_dram.name}_{nt}")
                        nc.sync.dma_start(wt[:], w_dram.ap()[nt])
                        wts.append(wt)
                        pss4.append([psb1.tile(
                            [128, 512], f32, tag="ps_b1",
                            name=f"ps_{w_dram.name}_{nt}_{ib}")
                            for ib in range(IB)])
                    for c in range(1, 8):
                        nc.sync.dma_start(
                            xt[c][:], xt_d.ap()[:, bass.ts(c, 4), :])
                    nc.sync.dma_start(cosT[:], cos_d.ap())
                    nc.sync.dma_start(sinTs[:], sin_d.ap())
                    nc.sync.dma_start(tri[:], tri_d.ap())
                    for kt in range(D_T):
                        for gi in range(len(g4)):
                            for ib in range(IB):
                                nc.tensor.matmul(
                                    pss4[gi][ib][:], wts[gi][:, kt, :],
                                    xts(kt, bass.ts(ib, 512)),
                                    start=(kt == 0), stop=(kt == D_T - 1))
                    for gi, (w_dram, nt, dst) in enumerate(g4):
                        for ib in range(IB):
                            rope_to(dst[:, bass.ts(ib, 512)], pss4[gi][ib][:],
                                    ib, f"{w_dram.name}{nt}_{ib}")

                    def proj_head(w_dram, nt, dst):
                        # dst[:, :] = rope(w[nt].T @ xt)
                        wt = wstr.tile([128, D_T, 128], bf16, tag="w32",
                                       name=f"w_{w_dram.name}_{nt}")
                        nc.sync.dma_start(wt[:], w_dram.ap()[nt])
                        ps = [psb1.tile([128, 512], f32, tag="ps_b1",
                                        name=f"ps_{w_dram.name}_{nt}_{ib}")
                              for ib in range(IB)]
                        for kt in range(D_T):
                            for ib in range(IB):
                                nc.tensor.matmul(
                                    ps[ib][:], wt[:, kt, :],
                                    xts(kt, bass.ts(ib, 512)),
                                    start=(kt == 0), stop=(kt == D_T - 1))
                        for ib in range(IB):
                            rope_to(dst[:, bass.ts(ib, 512)], ps[ib][:], ib,
                                    f"{w_dram.name}{nt}_{ib}")

                    for h in range(1, HL):
                        proj_head(wq_d, h, qT[h])
                    for h in range(LHL):
                        proj_head(wlkc_d, h, lkT[h])

                # ---- phase B2: token-major projections v, lv ----
                with tc.tile_pool(name="wvp", bufs=1) as wvp, \
                     tc.tile_pool(name="wlvp", bufs=1) as wlvp, \
                     tc.tile_pool(name="ps_b2", bufs=2, space="PSUM") as psb2:
                    wv_sb = wvp.tile([128, D_T, KVL * HD], bf16, tag="wv")
                    for c in range(4):
                        nc.sync.dma_start(
                            wv_sb[:, bass.ts(c, D_T // 4), :],
                            wv_d.ap()[:, bass.ts(c, D_T // 4), :])
                    for half in range(2):
                        wlv_sb = wlvp.tile([128, D_T, 512], bf16, tag="wlvh",
                                           name=f"wlvh_{half}")
                        hs = bass.ts(half, 512)
                        for c in range(4):
                            nc.sync.dma_start(
                                wlv_sb[:, bass.ts(c, D_T // 4), :],
                                wlv_d.ap()[:, bass.ts(c, D_T // 4), hs])
                        for tp_ in range(S_T // 2):   # token-tile pairs
                            pss = []
                            for u in range(2):
                                pair = [psb2.tile(
                                    [128, 512], f32, tag="ps_lv",
                                    name=f"ps_lv_{half}_{tp_}_{u}")]
                                if half == 0:
                                    pair.append(psb2.tile(
                                        [128, KVL * HD], f32, tag="ps_v",
                                        name=f"ps_v_{tp_}_{u}"))
                                pss.append(pair)
                            for kt in range(D_T):
                                st = kt == 0
                                sp = kt == D_T - 1
                                for u in range(2):
                                    tt = 2 * tp_ + u
                                    lhs = xts(kt, bass.ts(tt, 128))
                                    nc.tensor.matmul(pss[u][0][:], lhs,
                                                     wlv_sb[:, kt, :],
                                                     start=st, stop=sp)
                                    if half == 0:
                                        nc.tensor.matmul(pss[u][1][:], lhs,
                                                         wv_sb[:, kt, :],
                                                         start=st, stop=sp)
                            for u in range(2):
                                tt = 2 * tp_ + u
                                nc.any.tensor_copy(lv_sb[:, tt, hs],
                                                   pss[u][0][:])
                                if half == 0:
                                    nc.any.tensor_copy(v_sb[:, tt, :],
                                                       pss[u][1][:])

            # ---- phase C: attention (16 virtual heads, causal-trimmed) ----
            # ---- phase D: output projections (overlapped via scheduler) ---
            with tc.tile_pool(name="attnlat", bufs=1) as alp, \
                 tc.tile_pool(name="pt", bufs=6) as ptp, \
                 tc.tile_pool(name="rec", bufs=3) as recp, \
                 tc.tile_pool(name="wop", bufs=2) as wop, \
                 tc.tile_pool(name="ost", bufs=4) as ost, \
                 tc.tile_pool(name="ps_s", bufs=2, space="PSUM") as pss_, \
                 tc.tile_pool(name="ps_o", bufs=2, space="PSUM") as pso_, \
                 tc.tile_pool(name="ps_f", bufs=2, space="PSUM") as psf:

                attnT = [alp.tile([128, S], bf16, tag=f"attnT{h}",
                                  name=f"attnT{h}") for h in range(HL)]
                latT = [alp.tile([128, S], bf16, tag=f"latT{h}",
                                 name=f"latT{h}") for h in range(LHL)]

                # prefetch first o_proj weights; executes once SBUF frees
                wo2 = [None] * 4
                wlo2 = [None] * 4

                def load_np(np_):
                    wo2[np_] = wop.tile([128, HL, 1024], bf16, tag="wo2",
                                        name=f"wo2_{np_}")
                    wlo2[np_] = wop.tile([128, LHL, 1024], bf16, tag="wlo2",
                                         name=f"wlo2_{np_}")
                    for u in range(2):
                        nc.sync.dma_start(
                            wo2[np_][:, :, bass.ts(u, 512)],
                            wo_d.ap()[2 * np_ + u])
                        nc.sync.dma_start(
                            wlo2[np_][:, :, bass.ts(u, 512)],
                            wlo_d.ap()[2 * np_ + u])

                load_np(0)

                for vh in range(HL + LHL):
                    if vh < HL:
                        h = vh
                        ksrc = kT[h // GROUPS]
                        dst = attnT[h]
                        vof = (h // GROUPS) * HD
                        vtile = v_sb
                    else:
                        h = vh - HL
                        ksrc = lkT[h]
                        dst = latT[h]
                        vof = h * HD
                        vtile = lv_sb
                    qsrc = qT[h]

                    for ib in range(IB):
                        groups = _attn_blocks(ib)
                        pts = {}   # jb -> (pt_tile, off, width, sc)
                        denb = None
                        for gi, (blocks, width) in enumerate(groups):
                            ps_s = pss_.tile([128, 1024], f32, tag="ps_s",
                                             name=f"ps_s_{vh}_{ib}_{gi}")
                            if gi == len(groups) - 1:
                                denb = ps_s   # bank1 unused by scores: holds
                                              # the softmax denominator
                            for (jb, off, w, sc) in blocks:
                                nc.tensor.matmul(
                                    ps_s[:, off:off + w],
                                    ksrc[:, bass.ts(jb, 128)],
                                    qsrc[:, ib * 512 + sc: (ib + 1) * 512],
                                    start=True, stop=True)
                            pt = ptp.tile([128, 1024], bf16, tag="pt",
                                          name=f"pt_{vh}_{ib}_{gi}")
                            nc.scalar.activation(
                                pt[:, 0:width], ps_s[:, 0:width],
                                mybir.ActivationFunctionType.Exp,
                                scale=SCALE)
                            for (jb, off, w, sc) in blocks:
                                if jb >= 4 * ib:   # diagonal: mask leading 128
                                    nc.vector.tensor_tensor(
                                        pt[:, off:off + 128],
                                        pt[:, off:off + 128], tri[:],
                                        mybir.AluOpType.mult)
                                pts[jb] = (pt, off, w, sc)

                        njb = 4 * (ib + 1)
                        for jb in range(njb):
                            pt, off, w, sc = pts[jb]
                            nc.tensor.matmul(
                                denb[:, 512 + sc:1024], ones[:],
                                pt[:, off:off + w],
                                start=(jb == 0), stop=(jb == njb - 1),
                                skip_group_check=True)
                        rec = recp.tile([128, 512], f32, tag="rec",
                                        name=f"rec_{vh}_{ib}")
                        nc.vector.reciprocal_approx_fast(rec[:],
                                                         denb[:, 512:1024])
                        ps_o = pso_.tile([128, 512], f32, tag="ps_o",
                                         name=f"ps_o_{vh}_{ib}")
                        for jb in range(njb):
                            pt, off, w, sc = pts[jb]
                            nc.tensor.matmul(
                                ps_o[:, sc:512],
                                vtile[:, jb, vof:vof + HD],
                                pt[:, off:off + w],
                                start=(jb == 0), stop=(jb == njb - 1),
                                skip_group_check=True)
                        nc.vector.tensor_tensor(
                            dst[:, bass.ts(ib, 512)], ps_o[:], rec[:],
                            mybir.AluOpType.mult)

                # ---- phase D: out += attn @ w_o + lat @ w_lo (row-shard) --
                for np_ in range(4):       # pairs of 512-wide col blocks
                    if np_ + 1 < 4:
                        load_np(np_ + 1)
                    for tt in range(S_T):
                        ps0 = psf.tile([128, 512], f32, tag="ps_f",
                                       name=f"ps_f0_{np_}_{tt}")
                        ps1 = psf.tile([128, 512], f32, tag="ps_f",
                                       name=f"ps_f1_{np_}_{tt}")
                        for ui, ps in enumerate((ps0, ps1)):
                            us = bass.ts(ui, 512)
                            for h in range(HL):
                                nc.tensor.matmul(
                                    ps[:], attnT[h][:, bass.ts(tt, 128)],
                                    wo2[np_][:, h, us],
                                    start=(h == 0), stop=False)
                            for h in range(LHL):
                                nc.tensor.matmul(
                                    ps[:], latT[h][:, bass.ts(tt, 128)],
                                    wlo2[np_][:, h, us],
                                    start=False, stop=(h == LHL - 1))
                        for u, ps in enumerate((ps0, ps1)):
                            ot = ost.tile([128, 512], f32, tag="ot",
                                          name=f"ot_{np_}_{tt}_{u}")
                            nc.any.tensor_copy(ot[:], ps[:])
                            nc.sync.dma_start(
                                out_ap[:, tt, bass.ds(
                                    (2 * np_ + u) * 512, 512)],
                                ot[:])

    nc.compile()
    return nc


_NC = None


def _get_program():
    global _NC
    if _NC is None:
        _NC = _build_program()
    return _NC


def _rope_tables():
    inv_freq = 1.0 / (THETA ** (np.arange(0, HD, 2, dtype=np.float32) / HD))
    t = np.arange(S, dtype=np.float32)
    freqs = np.outer(t, inv_freq)                       # [S, 64]
    emb = np.concatenate([freqs, freqs], axis=-1)       # [S, HD]
    cosT = np.cos(emb).T.astype(BF16).copy()            # [HD, S]
    sinT = np.sin(emb).T.astype(np.float32)
    sinTs = np.concatenate([-sinT[:HD // 2], sinT[HD // 2:]], 0).astype(
        BF16).copy()
    return cosT, sinTs


def _tri_mask():
    # tri[p, s] = 1.0 iff p <= s (k-token p attends-allowed for query col s)
    p = np.arange(128)[:, None]
    s = np.arange(128)[None, :]
    return (p <= s).astype(BF16)


def _tile_w_fm(w, n_tiles, kt):
    # [K, n_tiles*128] -> [n_tiles, 128(p of K), kt, 128]
    K, N = w.shape
    assert K == kt * 128 and N == n_tiles * 128
    return np.ascontiguousarray(
        np.asarray(w, dtype=np.float32).reshape(
            kt, 128, n_tiles, 128).transpose(2, 1, 0, 3)).astype(BF16)


def _tile_w_tm(w, kt):
    # [K, N] -> [128(p of K), kt, N]
    K, N = w.shape
    assert K == kt * 128
    return np.ascontiguousarray(
        np.asarray(w, dtype=np.float32).reshape(
            kt, 128, N).transpose(1, 0, 2)).astype(BF16)


def _tile_w_out(w):
    # [1024, D] -> [8(nb), 128(p of rows), 8(h), 512]
    return np.ascontiguousarray(
        w.reshape(8, 128, D // 512, 512).transpose(2, 1, 0, 3)).astype(BF16)


def _prep_core_inputs(inputs, c):
    """Build the per-core input map for core c from full fp32 inputs."""
    cosT, sinTs = _rope_tables()
    b, g = divmod(c, TP)
    x = np.asarray(inputs["hidden_states"][b], dtype=np.float32)   # [S, D]
    xt = np.ascontiguousarray(
        x.T.reshape(D_T, 128, S).transpose(1, 0, 2)).astype(BF16)
    qs = slice(g * HL * HD, (g + 1) * HL * HD)
    kvs = slice(g * KVL * HD, (g + 1) * KVL * HD)
    ls = slice(g * LHL * HD, (g + 1) * LHL * HD)
    wlkc = np.asarray(inputs["w_lq"], dtype=np.float32) @ np.asarray(
        inputs["w_lk"], dtype=np.float32)[:, ls]
    return {
        "xt": xt,
        "wq": _tile_w_fm(np.asarray(inputs["w_q"])[:, qs], HL, D_T),
        "wk": _tile_w_fm(np.asarray(inputs["w_k"])[:, kvs], KVL, D_T),
        "wv": _tile_w_tm(np.asarray(inputs["w_v"])[:, kvs], D_T),
        "wlkc": _tile_w_fm(wlkc, LHL, D_T),
        "wlv": _tile_w_tm(np.asarray(inputs["w_lv"])[:, ls], D_T),
        "wo": _tile_w_out(np.asarray(inputs["w_o"])[qs, :]),
        "wlo": _tile_w_out(np.asarray(inputs["w_lo"])[ls, :]),
        "cosT": cosT,
        "sinTs": sinTs,
        "trimask": _tri_mask(),
    }


def kernel(hidden_states, w_q, w_k, w_v, w_o, w_lq, w_lk, w_lv, w_lo):
    nc = _get_program()
    inputs = {
        "hidden_states": hidden_states, "w_q": w_q, "w_k": w_k, "w_v": w_v,
        "w_o": w_o, "w_lq": w_lq, "w_lk": w_lk, "w_lv": w_lv, "w_lo": w_lo,
    }
    in_maps = [_prep_core_inputs(inputs, c) for c in range(NCORES)]
    res = run_bass_kernel_spmd(nc, in_maps, list(range(NCORES))).results

    out = np.zeros((B, S, D), dtype=np.float32)
    for c in range(NCORES):
        out[c // TP] += res[c]["out"]
    return out
